# revision 31
# baseline (speedup 1.0000x reference)
"""Trainium2 Bass kernel for the CMPO3/GTN tensor-train contraction model.

Math (reference): three tensor-train chains over L=64 sites, each site
contracted with per-site input vectors derived from reductions of x:
  vpx[i,b,:] = mean_ch  x[b,i,:,:]   (PIX-dim vectors)
  vch[i,b,:] = mean_pix x[b,i,:,:]   (CH-dim vectors)
  psi chain (bond 64, phys PIX) -> scalar per batch
  chi chain (bond 32, phys CH)  -> (batch, 10)
  phi chain (bond 64, one-hot phys) -> global scalar
  out = chi_out * (psi_val * phi_val)[:, None]

Strategy (2 SPMD launches over 8 cores):
  Launch A (site/patch-sharded): each core owns 8 patches of x and the
    matching slices of psi_mid/chi_mid.  x is host-packed per site as
    [128 = (ch, p_hi), 8192 = (p_lo, b)] so BOTH data reductions run on
    the PE as matmuls against constant 0/1 selection matrices (E sums the
    ch lanes -> vpx, F sums over (ch,p_hi) with a 32-matmul accumulation
    group over p_lo -> vch); the 1/CH, 1/PIX mean scales are folded into
    E and F.  Cheap f16 PE transposes put vpxT/vchT into the
    p-on-partition layout the transfer-matrix matmuls need (psi weights
    are host-permuted to the matching (p_lo, p_hi) order).  The per-site
    transfer matrices
      M_s[b][l,r] = sum_p W_s[l,r,p] * u_s[b,p]
    go to DRAM as (site, b, l*r) f16.  PSUM->SBUF drains run on Act+DVE
    (the only engines allowed to read PSUM); DMAs are spread across
    SP/Act/Pool so the PE (~69us of matmul rows) stays the binding
    engine.  Boundary vectors are computed on the cores owning patch
    0 / 63.
  Launch B (batch-sharded): each core contracts the chains for its 32
    samples as four independent streams (psi fwd/bwd, chi fwd/bwd), each
    a sequence of per-batch stationary matvecs on the PE.  The
    batch-independent phi chain rides along as column 32 of the psi
    streams (its per-site 64x64 matrices are just one more stationary),
    so phi costs one extra matvec per site instead of a serial chain in
    launch A.  Stream state copies run on DVE/Act; M tile DMAs are
    spread across SP/Act/Pool.

All host-side work is layout glue only (transposes/slices/concats/dtype
casts plus folding the 1/CH, 1/PIX mean scales into constant selection
matrices).
"""

import sys

import numpy as np

if "/opt/trn_rl_repo" not in sys.path:
    sys.path.insert(0, "/opt/trn_rl_repo")

import concourse.bass as bass
import concourse.bacc as bacc
import concourse.mybir as mybir
import concourse.tile as tile
from concourse.bass_utils import run_bass_kernel_spmd

F32 = mybir.dt.float32
F16 = mybir.dt.float16
AX = mybir.AxisListType
ADD = mybir.AluOpType.add
MULT = mybir.AluOpType.mult

L, CH, PIX, PAT, RC, BD, OUT, B = 64, 16, 256, 64, 32, 64, 10, 256
NCORES = 8
SLOTS = 8          # patches per core in launch A
BSH = B // NCORES  # batch per core in launch B (32)
BW = BSH + 1       # psi stream width in launch B (batches + phi column)
NMID = L - 2       # 62 mid sites
NPF = 32           # psi fwd sites (mids 0..31)
NPB = 30           # psi bwd sites (mids 61..32)
NCF = 31           # chi fwd sites (mids 0..30)
NCB = 31           # chi bwd sites (mids 61..31)
PTF, PTB = NPF // 2, NPB // 2        # psi tiles per direction (2 sites/tile)
CTF, CTB = (NCF + 2) // 3, (NCB + 2) // 3  # chi tiles (3 sites/tile)
PGRP = 2           # psi tiles per DMA (after a small first group)
CGRP = 2           # chi tiles per DMA (after a small first group)

# p' permutation: the E-matmul/transpose pipeline yields vpxT rows ordered
# p' = p_lo*8 + p_hi where p = p_hi*32 + p_lo; psi weights (p-indexed) are
# host-permuted with PPERM so device contractions line up.
PPERM = np.array([(pp % 8) * 32 + pp // 8 for pp in range(PIX)])


# ---------------------------------------------------------------- launch A
def build_launch_a():
    nc = bacc.Bacc("TRN2", target_bir_lowering=False, debug=False,
                   num_devices=NCORES)
    # x per site: [128 = (ch, p_hi), (p_lo, b)] f16
    xe_in = nc.dram_tensor("xe", [SLOTS, 128, 64 * 128], F16, kind="ExternalInput").ap()
    # psi mids, p' permuted rows: (site, p', l*r)
    wpsi_in = nc.dram_tensor("wpsi", [SLOTS, PIX, BD * BD], F16, kind="ExternalInput").ap()
    # chi mids packed 3 slots/tile at 32-partition strides: (3, 96, rc*rc)
    wchi_in = nc.dram_tensor("wchi", [3, 96, RC * RC], F16, kind="ExternalInput").ap()
    # constant selection matrices (scales folded)
    emat_in = nc.dram_tensor("emat", [128, 8], F16, kind="ExternalInput").ap()
    fmat_in = nc.dram_tensor("fmat", [128, CH], F16, kind="ExternalInput").ap()
    wfp_in = nc.dram_tensor("wfp", [PIX, BD], F16, kind="ExternalInput").ap()
    wlp_in = nc.dram_tensor("wlp", [PIX, BD], F16, kind="ExternalInput").ap()
    wfc_in = nc.dram_tensor("wfc", [CH, RC], F16, kind="ExternalInput").ap()
    wlc_in = nc.dram_tensor("wlc", [CH, RC * OUT], F16, kind="ExternalInput").ap()
    ident_in = nc.dram_tensor("ident", [128, 128], F16, kind="ExternalInput").ap()

    mpsi_out = nc.dram_tensor("mpsi", [SLOTS, B, BD * BD], F16, kind="ExternalOutput").ap()
    mchi_out = nc.dram_tensor("mchi", [SLOTS, B, RC * RC], F16, kind="ExternalOutput").ap()
    v0p_out = nc.dram_tensor("v0p", [B, BD], F32, kind="ExternalOutput").ap()
    v0c_out = nc.dram_tensor("v0c", [B, RC], F32, kind="ExternalOutput").ap()
    wlast_out = nc.dram_tensor("wlast", [B, BD], F32, kind="ExternalOutput").ap()
    tchi_out = nc.dram_tensor("tchi", [B, RC * OUT], F32, kind="ExternalOutput").ap()

    with tile.TileContext(nc) as tc:
        with (
            tc.tile_pool(name="consts", bufs=1) as cpool,
            tc.tile_pool(name="xw", bufs=3) as xwpool,
            tc.tile_pool(name="vecs", bufs=2) as vpool,
            tc.tile_pool(name="mstage", bufs=2) as mpool,
            tc.tile_pool(name="small", bufs=2) as spool,
            tc.tile_pool(name="psmm", bufs=2, space="PSUM") as psmm,
            tc.tile_pool(name="pschi", bufs=1, space="PSUM") as pschi,
            tc.tile_pool(name="pssm", bufs=1, space="PSUM") as pssm,
        ):
            ident16 = cpool.tile([128, 128], F16, name="ident16")
            nc.sync.dma_start(out=ident16, in_=ident_in)
            emat = cpool.tile([128, 8], F16, name="emat")
            nc.sync.dma_start(out=emat, in_=emat_in)
            fmat = cpool.tile([128, CH], F16, name="fmat")
            nc.sync.dma_start(out=fmat, in_=fmat_in)
            # chi weights: slot s lives at rows 32*(s%3)..+16 of tile s//3
            wc_t = cpool.tile([96, 3, RC * RC], F16, name="wc_t")
            for t in range(3):
                nc.scalar.dma_start(out=wc_t[:, t, :], in_=wchi_in[t])
            # vchT accumulates per slot at rows 32*(s%3)..+16 of tile s//3
            vchT = cpool.tile([96, 3, B], F16, name="vchT")

            # boundary weights (p' permuted on host for the psi ones)
            wfp_t = cpool.tile([128, 2 * BD], F16, name="wfp_t")
            wlp_t = cpool.tile([128, 2 * BD], F16, name="wlp_t")
            for k in range(2):
                nc.scalar.dma_start(out=wfp_t[:, k * BD:(k + 1) * BD],
                                    in_=wfp_in[k * 128:(k + 1) * 128, :])
                nc.scalar.dma_start(out=wlp_t[:, k * BD:(k + 1) * BD],
                                    in_=wlp_in[k * 128:(k + 1) * 128, :])
            wfc_t = cpool.tile([CH, RC], F16, name="wfc_t")
            nc.scalar.dma_start(out=wfc_t, in_=wfc_in)
            wlc_t = cpool.tile([CH, RC * OUT], F16, name="wlc_t")
            nc.scalar.dma_start(out=wlc_t, in_=wlc_in)

            # boundary slots (0 on core 0, 7 on core 7) processed first to
            # shorten the tail; M writes for them land early too.
            ORDER = [0, SLOTS - 1] + list(range(1, SLOTS - 1))
            xe_tiles = {}
            wp_tiles = {}

            def fetch_xe(slot, early=False):
                t = xwpool.tile([128, 64 * 128], F16, name="xe_t",
                                tag="xe", bufs=3)
                engs = ([nc.gpsimd, nc.sync, nc.scalar, nc.gpsimd] if early
                        else [nc.sync, nc.scalar, nc.sync, nc.scalar])
                for q4 in range(4):
                    engs[q4].dma_start(
                        out=t[:, 2048 * q4:2048 * (q4 + 1)],
                        in_=xe_in[slot, :, 2048 * q4:2048 * (q4 + 1)])
                xe_tiles[slot] = t

            def fetch_wp(slot):
                t = xwpool.tile([128, 2, BD * BD], F16, name="wp",
                                tag="wp", bufs=3)
                nc.gpsimd.dma_start(out=t[:, 0, 0:2048],
                                    in_=wpsi_in[slot, 0:128, 0:2048])
                nc.gpsimd.dma_start(out=t[:, 0, 2048:4096],
                                    in_=wpsi_in[slot, 0:128, 2048:4096])
                nc.gpsimd.dma_start(out=t[:, 1, 0:2048],
                                    in_=wpsi_in[slot, 128:256, 0:2048])
                nc.sync.dma_start(out=t[:, 1, 2048:3072],
                                  in_=wpsi_in[slot, 128:256, 2048:3072])
                nc.gpsimd.dma_start(out=t[:, 1, 3072:4096],
                                    in_=wpsi_in[slot, 128:256, 3072:4096])
                wp_tiles[slot] = t

            fetch_xe(ORDER[0])
            fetch_xe(ORDER[1], early=True)
            fetch_wp(ORDER[0])
            fetch_wp(ORDER[1])
            stage1_out = {}

            def stage1(slot, oi):
                m3 = 32 * (slot % 3)   # vchT/wc partition base
                t3 = slot // 3
                if oi + 2 < SLOTS:
                    fetch_xe(ORDER[oi + 2])
                    fetch_wp(ORDER[oi + 2])
                xe_t = xe_tiles.pop(slot)

                # -------- data reductions on the PE
                # vpx: 64 E-matmuls -> cols 0:512 of a shared psum tile
                # (bank 0); vch F-accumulation lands in cols 512:544 (bank 1)
                pEF = psmm.tile([128, 1024], F32, name="pEF", tag="psEF",
                                bufs=1)
                pE = pEF[:, 0:512]
                pF = pEF[:, 512:512 + 2 * CH]
                for j in range(64):
                    bh = j % 2
                    pl = j // 2
                    sl = xe_t[:, 128 * j:128 * (j + 1)]
                    # one start/stop per psum bank: bank 0 holds all E
                    # outputs, bank 1 the F accumulators.  Disjoint writes
                    # after the start land on pending-zero bytes.
                    nc.tensor.matmul(pE[:, 256 * bh + 8 * pl:
                                        256 * bh + 8 * (pl + 1)], sl, emat,
                                     start=(j == 0), stop=(j == 63))
                    nc.tensor.matmul(pF[:, CH * bh:CH * (bh + 1)], sl, fmat,
                                     start=(j == 0), stop=(j == 63))
                v_t = vpool.tile([128, 2 * B], F16, name="V", tag="V")
                with nc.allow_low_precision(reason="vpx f16"):
                    nc.vector.tensor_copy(out=v_t, in_=pE)
                V = [v_t[:, 0:B], v_t[:, B:2 * B]]
                # transpose V chunks -> U[pc][128 p', 256 b]
                U = [vpool.tile([128, B], F16, name=f"U{pc}", tag=f"U{pc}")
                     for pc in range(2)]
                ptt = pssm.tile([128, 1024], F16, name="ptt", tag="pst",
                                bufs=1)
                for pc in range(2):
                    for bh in range(2):
                        tps = ptt[:, 128 * (2 * pc + bh):
                                  128 * (2 * pc + bh + 1)]
                        nc.tensor.transpose(
                            tps, V[bh][:, 128 * pc:128 * (pc + 1)], ident16)
                    nc.vector.tensor_copy(
                        out=U[pc], in_=ptt[:, 256 * pc:256 * (pc + 1)])
                # vch: copy + transpose into vchT rows m3..m3+16
                c_s = spool.tile([128, 2 * CH], F16, name="c_s", tag="c_s")
                with nc.allow_low_precision(reason="vch f16"):
                    nc.vector.tensor_copy(out=c_s, in_=pF)
                for bh in range(2):
                    tpc = ptt[0:CH, 512 + 128 * bh:512 + 128 * (bh + 1)]
                    nc.tensor.transpose(tpc, c_s[:, CH * bh:CH * (bh + 1)],
                                        ident16)
                nc.vector.tensor_copy(
                    out=vchT[m3:m3 + CH, t3, :], in_=ptt[0:CH, 512:768])
                stage1_out[slot] = U

            def stage2(slot, oi):
                m3 = 32 * (slot % 3)
                t3 = slot // 3
                U = stage1_out.pop(slot)
                # -------- psi mid transfer matrices
                wp = wp_tiles.pop(slot)
                mst = mpool.tile([128, 2, BD * BD], F16, name="mst", tag="mst")
                # 16 psum chunks per slot ([128,512], bufs=4) so the PE can
                # run several chunks ahead of the Act/DVE drains
                for bh in range(2):
                    for n in range(8):
                        ps = psmm.tile([128, 512], F32, name="ps",
                                       tag="ps_mm", bufs=3)
                        cs = slice(n * 512, (n + 1) * 512)
                        nc.tensor.matmul(
                            ps, U[0][:, bh * 128:(bh + 1) * 128],
                            wp[:, 0, cs], start=True, stop=False)
                        nc.tensor.matmul(
                            ps, U[1][:, bh * 128:(bh + 1) * 128],
                            wp[:, 1, cs], start=False, stop=True)
                        with nc.allow_low_precision(reason="m f16"):
                            if (n + bh) % 2:
                                nc.scalar.copy(
                                    mst[:, bh, cs], ps)
                            else:
                                nc.vector.tensor_copy(
                                    out=mst[:, bh, cs], in_=ps)
                    if oi == SLOTS - 1:
                        mqs = ([nc.gpsimd, nc.scalar] if bh == 0 else
                               [nc.sync, nc.gpsimd])
                    else:
                        mqs = ([nc.gpsimd, nc.gpsimd] if bh == 0 else
                               [nc.sync, nc.sync])
                    for hh in range(2):
                        mqs[hh].dma_start(
                            out=mpsi_out[slot, bh * 128:(bh + 1) * 128,
                                         2048 * hh:2048 * (hh + 1)],
                            in_=mst[:, bh, 2048 * hh:2048 * (hh + 1)])

                # -------- chi mid transfer matrices
                mstc = mpool.tile([128, 2, RC * RC], F16, name="mstc",
                                  tag="mstc")
                for bh in range(2):
                    psc = pschi.tile([128, 1024], F32, name="psc",
                                     tag="ps_chi", bufs=1)
                    for n in range(2):
                        nc.tensor.matmul(
                            psc[:, n * 512:(n + 1) * 512],
                            vchT[m3:m3 + CH, t3, bh * 128:(bh + 1) * 128],
                            wc_t[m3:m3 + CH, t3, n * 512:(n + 1) * 512],
                            start=True, stop=True)
                    with nc.allow_low_precision(reason="mc f16"):
                        if bh:
                            nc.scalar.copy(mstc[:, bh, :], psc)
                        else:
                            nc.vector.tensor_copy(out=mstc[:, bh, :], in_=psc)
                nc.sync.dma_start(out=mchi_out[slot].rearrange(
                    "(c b) f -> b c f", c=2), in_=mstc)

                # -------- boundary contractions (host keeps core0/core7 only)
                if slot in (0, SLOTS - 1):
                    # base-0 copy of this slot's vchT for the boundary matmuls
                    vch0 = spool.tile([CH, B], F16, name="vch0", tag="vch0")
                    nc.gpsimd.tensor_copy(out=vch0,
                                          in_=vchT[m3:m3 + CH, t3, :])
                    wpt = wfp_t if slot == 0 else wlp_t
                    pout = v0p_out if slot == 0 else wlast_out
                    cout = v0c_out if slot == 0 else tchi_out
                    wct_b = wfc_t if slot == 0 else wlc_t
                    cw = RC if slot == 0 else RC * OUT
                    for bh in range(2):
                        psbt = psmm.tile([128, 512], F32, name="ps",
                                         tag="ps_mm", bufs=3)
                        psb = psbt[:, 0:BD]
                        for k in range(2):
                            nc.tensor.matmul(psb,
                                             U[k][:, bh * 128:(bh + 1) * 128],
                                             wpt[:, k * BD:(k + 1) * BD],
                                             start=(k == 0), stop=(k == 1))
                        v0s = spool.tile([128, BD], F32, name="v0s", tag="bnd")
                        nc.vector.tensor_copy(out=v0s, in_=psb)
                        nc.gpsimd.dma_start(out=pout[bh * 128:(bh + 1) * 128, :],
                                            in_=v0s)
                        psct = psmm.tile([128, 512], F32, name="ps",
                                         tag="ps_mm", bufs=3)
                        psc0 = psct[:, 0:cw]
                        nc.tensor.matmul(psc0, vch0[:, bh * 128:(bh + 1) * 128],
                                         wct_b, start=True, stop=True)
                        v0cs = spool.tile([128, cw], F32, name="v0cs",
                                          tag="bndc")
                        nc.vector.tensor_copy(out=v0cs, in_=psc0)
                        nc.gpsimd.dma_start(out=cout[bh * 128:(bh + 1) * 128, :],
                                            in_=v0cs)

            stage1(ORDER[0], 0)
            for oi in range(SLOTS):
                if oi + 1 < SLOTS:
                    stage1(ORDER[oi + 1], oi + 1)
                stage2(ORDER[oi], oi)
    nc.finalize()
    return nc


# ---------------------------------------------------------------- launch B
def build_launch_b():
    """Batch-sharded chains as four per-batch stationary-matvec streams.

    Each stream holds its state as an f16 [bond, width] SBUF tile whose
    partition base cycles with the site index (psi: 0/64; chi: 0/32/64),
    matching where the host packed that site's stationary matrix in its
    DMA tile (matmul requires lhsT/rhs/psum bases to agree and be in
    {0,32,64}).  A site = `width` single-column matmuls (one per batch,
    PSUM column out) + one PSUM->SBUF f16 state copy (DVE/Act).  The psi
    streams carry the batch-independent phi chain as column 32.  The chi
    bwd stream carries a matrix state (32l x 10o per batch).  Finals: the
    psi/phi dot via a ones-matmul partition reduce; chi fwd/bwd per-batch
    dots to [10, 32b], transposed and scaled by psi*phi on the DVE.
    """
    nc = bacc.Bacc("TRN2", target_bir_lowering=False, debug=False,
                   num_devices=NCORES)
    mpf_in = nc.dram_tensor("mpf", [PTF, 128, BW * BD], F16, kind="ExternalInput").ap()
    mpb_in = nc.dram_tensor("mpb", [PTB, 128, BW * BD], F16, kind="ExternalInput").ap()
    mcf_in = nc.dram_tensor("mcf", [CTF, 96, BSH * RC], F16, kind="ExternalInput").ap()
    mcb_in = nc.dram_tensor("mcb", [CTB, 96, BSH * RC], F16, kind="ExternalInput").ap()
    # packed initial states: cols 0:33 v0pT|phi0, 33:66 wlT|phiL (rows 0:64);
    # cols 66:98 v0cT (rows 0:32), cols 98:418 tT (rows 0:32)
    NI = 2 * BW + BSH + BSH * OUT
    init_in = nc.dram_tensor("init", [BD, NI], F16, kind="ExternalInput").ap()
    ident_in = nc.dram_tensor("ident", [RC, RC], F32, kind="ExternalInput").ap()

    out_out = nc.dram_tensor("out", [BSH, OUT], F32, kind="ExternalOutput").ap()

    with tile.TileContext(nc) as tc:
        with (
            tc.tile_pool(name="consts", bufs=1) as cpool,
            tc.tile_pool(name="mload", bufs=2) as mpool,
            tc.tile_pool(name="states", bufs=2) as spool,
            tc.tile_pool(name="psA", bufs=1, space="PSUM") as psA,
            tc.tile_pool(name="psB", bufs=1, space="PSUM") as psB,
        ):
            ident_t = cpool.tile([RC, RC], F32, name="ident_t")
            nc.gpsimd.dma_start(out=ident_t, in_=ident_in)
            ones32 = cpool.tile([128, 1], F32, name="ones32")
            nc.vector.memset(ones32, 1.0)
            onesw = cpool.tile([128, BSH], F32, name="onesw")
            nc.vector.memset(onesw, 1.0)

            # stream initial states, one packed DMA
            init_t = cpool.tile([BD, NI], F16, name="init_t")
            nc.sync.dma_start(out=init_t, in_=init_in)
            stf = init_t[0:BD, 0:BW]
            stb = init_t[0:BD, BW:2 * BW]
            stc = init_t[0:RC, 2 * BW:2 * BW + BSH]
            stg = init_t[0:RC, 2 * BW + BSH:NI]

            # group DMA tiles for the four streams: one shared weighted
            # rotation over the three DMA-capable queues (DVE can't DMA);
            # Act gets a lighter share since it also drains chi-bwd states
            ROT = [nc.sync, nc.gpsimd, nc.sync, nc.gpsimd, nc.scalar]
            _gctr = {"n": 0}

            def load_group(tag, dram, t0, ntiles, width):
                gt = mpool.tile([dram.shape[1], ntiles, width], F16,
                                name=f"g_{tag}", tag=f"g_{tag}", bufs=3)
                q = ROT[_gctr["n"] % len(ROT)]
                _gctr["n"] += 1
                q.dma_start(
                    out=gt, in_=dram[t0:t0 + ntiles].rearrange("t p f -> p t f"))
                return gt

            # Each stream is a generator yielding once per site so the four
            # chains can be emitted interleaved (round-robin): the PE executes
            # its queue in program order, so sequential emission would
            # serialize the streams' latencies.
            def stream_steps(tag, dram, nsites, state, ps_pool,
                             bond, per_tile, grp, owidth, ncols, result,
                             copy_engines):
                gt = None
                ntiles_tot = (nsites + per_tile - 1) // per_tile
                # group boundaries: first group small (2) so the stream can
                # start as soon as possible; then groups of `grp`
                bounds = [0, min(1, ntiles_tot)]
                while bounds[-1] < ntiles_tot:
                    bounds.append(min(bounds[-1] + grp, ntiles_tot))
                tile2group = {}
                for gi in range(len(bounds) - 1):
                    for t in range(bounds[gi], bounds[gi + 1]):
                        tile2group[t] = (gi, bounds[gi], t - bounds[gi])
                for s in range(nsites):
                    t_idx, off = divmod(s, per_tile)
                    gi, g0, g_off = tile2group[t_idx]
                    if t_idx == g0 and off == 0:
                        n = bounds[gi + 1] - g0
                        gt = load_group(tag, dram, g0, n, ncols * bond)
                    base = bond * off
                    nbase = bond * ((s + 1) % per_tile)
                    ps = ps_pool.tile([128, ncols * owidth], F32,
                                      name=f"ps_{tag}", tag=f"ps_{tag}",
                                      bufs=1)
                    for b in range(ncols):
                        nc.tensor.matmul(
                            ps[nbase:nbase + bond, owidth * b:owidth * (b + 1)],
                            gt[base:base + bond, g_off,
                               bond * b:bond * (b + 1)],
                            state[base:base + bond,
                                  owidth * b:owidth * (b + 1)],
                            start=True, stop=True)
                    state = spool.tile([128, ncols * owidth], F16,
                                       name=f"st_{tag}", tag=tag)
                    ceng = copy_engines[s % len(copy_engines)]
                    with nc.allow_low_precision(reason="f16 chain state"):
                        if ceng is nc.scalar:
                            ceng.copy(state[nbase:nbase + bond, :],
                                      ps[nbase:nbase + bond, :])
                        else:
                            ceng.tensor_copy(out=state[nbase:nbase + bond, :],
                                             in_=ps[nbase:nbase + bond, :])
                    yield
                result.append(state)

            res_f, res_b, res_c, res_g = [], [], [], []
            gens = [
                stream_steps("stf", mpf_in, NPF, stf, psA,
                             BD, 2, PGRP, 1, BW, res_f, [nc.vector]),
                stream_steps("stb", mpb_in, NPB, stb, psA,
                             BD, 2, PGRP, 1, BW, res_b, [nc.scalar]),
                stream_steps("stc", mcf_in, NCF, stc, psB,
                             RC, 3, CGRP, 1, BSH, res_c, [nc.vector]),
                stream_steps("stg", mcb_in, NCB, stg, psB,
                             RC, 3, CGRP, OUT, BSH, res_g,
                             [nc.scalar, nc.vector]),
            ]
            live = list(gens)
            while live:
                for g in list(live):
                    try:
                        next(g)
                    except StopIteration:
                        live.remove(g)
            stf, stb, stc, stg = res_f[0], res_b[0], res_c[0], res_g[0]

            fb_f = BD * (NPF % 2)   # 0
            fb_b = BD * (NPB % 2)   # 0
            fb_c = RC * (NCF % 3)   # 32
            fb_g = RC * (NCB % 3)   # 32

            # psi_val[b] = sum_l stf[l,b]*stb[l,b]; col 32 gives phi_val.
            # f32 throughout: the products are ~1e-8 and underflow in f16.
            prod = spool.tile([128, BW], F32, name="prod", tag="prod")
            nc.vector.tensor_tensor(out=prod[fb_f:fb_f + BD, :],
                                    in0=stf[fb_f:fb_f + BD, :],
                                    in1=stb[fb_b:fb_b + BD, :],
                                    op=MULT)
            ppv = psA.tile([BW, 1], F32, name="ppv", tag="ppv", bufs=1)
            nc.tensor.matmul(ppv, prod[fb_f:fb_f + BD, :],
                             ones32[fb_f:fb_f + BD, :], start=True, stop=True)
            psiphi = spool.tile([BW, 1], F32, name="psiphi", tag="fin")
            nc.vector.tensor_copy(out=psiphi, in_=ppv)
            # replicate phival (row 32) across the 32 batch partitions via a
            # ones-row matmul (bases must agree at 32), then fold into psival
            prep = psA.tile([BSH, 1], F32, name="prep", tag="prep", bufs=1)
            nc.tensor.matmul(prep, onesw[32:33, :], psiphi[BSH:BW, 0:1],
                             start=True, stop=True)
            scal = spool.tile([BSH, 1], F32, name="scal", tag="fin2c")
            nc.vector.tensor_tensor(out=scal, in0=psiphi[0:BSH, :],
                                    in1=prep, op=MULT)

            # chi_out[o,b] = sum_l stg[l, b*OUT+o] * stc[l, b]
            pcf = psB.tile([OUT, BSH], F32, name="pcf", tag="pcf", bufs=1)
            for b in range(BSH):
                nc.tensor.matmul(pcf[:, b:b + 1],
                                 stg[fb_g:fb_g + RC, OUT * b:OUT * (b + 1)],
                                 stc[fb_c:fb_c + RC, b:b + 1],
                                 start=True, stop=True)
            chifs = spool.tile([OUT, BSH], F32, name="chifs", tag="fin2")
            nc.vector.tensor_copy(out=chifs, in_=pcf)
            pt = psA.tile([BSH, OUT], F32, name="pt", tag="pt", bufs=1)
            nc.tensor.transpose(pt, chifs, ident_t[0:OUT, 0:OUT])
            res = spool.tile([BSH, OUT], F32, name="res", tag="fin3")
            nc.vector.tensor_scalar_mul(out=res, in0=pt, scalar1=scal)
            nc.sync.dma_start(out=out_out, in_=res)
    nc.finalize()
    return nc


# ------------------------------------------------------------- host glue
_cache = {}
LAST_RESULTS = []  # [(label, BassKernelResults)] from the most recent kernel()
LAST_INMAPS = {}   # {"a": in_maps_a, "b": in_maps_b} from the most recent kernel()


def _prep_inputs_a(inputs):
    # f16 upload of x: the on-device reductions accumulate in f32; the
    # 0.05% per-element cast error is far below the f16 weight error.
    x = np.asarray(inputs["x"], dtype=np.float32)
    # [site, (ch, p_hi), (p_lo, b)]: x[b, site, p, c] with p = ph*32 + pl
    xe = np.ascontiguousarray(
        x.transpose(1, 3, 2, 0)                       # (site, c, p, b)
        .reshape(PAT, CH, 8, 32, B)                   # (site, c, ph, pl, b)
        .reshape(PAT, 128, 32 * B).astype(np.float16))

    # psi_mid (62,l,r,p) -> (62, p', l*r), rows p'-permuted (no /CH: E has it)
    pm = inputs["psi_mid"].astype(np.float32)
    wpsi = np.ascontiguousarray(
        pm.transpose(0, 3, 1, 2).reshape(NMID, PIX, BD * BD)[:, PPERM, :])
    # chi_mid (62,l,r,ch) -> (62, ch, rc*rc) (no /PIX: F has it)
    cm = inputs["chi_mid"].astype(np.float32)
    wchi = np.ascontiguousarray(
        cm.transpose(0, 3, 1, 2).reshape(NMID, CH, RC * RC))

    wfp = np.ascontiguousarray(
        inputs["psi_first"].T.astype(np.float32)[PPERM, :]).astype(np.float16)
    wlp = np.ascontiguousarray(
        inputs["psi_last"].T.astype(np.float32)[PPERM, :]).astype(np.float16)
    wfc = np.ascontiguousarray(
        inputs["chi_first"].T.astype(np.float32)).astype(np.float16)
    wlc = np.ascontiguousarray(
        inputs["chi_last"].astype(np.float32).transpose(1, 0, 2)
        .reshape(CH, RC * OUT)).astype(np.float16)

    ident = np.eye(128, dtype=np.float16)

    # selection matrices: rows (c, ph); E sums over c (vpx = mean_c),
    # F sums over ph (+ the p_lo accumulation group) for vch = mean_p.
    emat = np.zeros((128, 8), np.float16)
    fmat = np.zeros((128, CH), np.float16)
    for q in range(128):
        c, ph = q // 8, q % 8
        emat[q, ph] = 1.0 / CH
        fmat[q, c] = 1.0 / PIX

    zero_pw = np.zeros_like(wpsi[0])
    zero_cw = np.zeros_like(wchi[0])
    in_maps = []
    for k in range(NCORES):
        # slot j of core k handles patch 8k+j; mid site s uses weight s-1
        wp_slots = np.stack([
            wpsi[8 * k + j - 1] if 1 <= 8 * k + j <= NMID else zero_pw
            for j in range(SLOTS)]).astype(np.float16)
        wc_slots = np.stack([
            wchi[8 * k + j - 1] if 1 <= 8 * k + j <= NMID else zero_cw
            for j in range(SLOTS)]).astype(np.float16)
        # pack chi weights: tile t rows 32*m..+16 = slot 3t+m
        wc_packed = np.zeros((3, 96, RC * RC), np.float16)
        for j in range(SLOTS):
            wc_packed[j // 3, 32 * (j % 3):32 * (j % 3) + CH] = wc_slots[j]
        z = np.zeros
        in_maps.append({
            "xe": np.ascontiguousarray(xe[8 * k:8 * (k + 1)]),
            "wpsi": np.ascontiguousarray(wp_slots),
            "wchi": wc_packed,
            "emat": emat,
            "fmat": fmat,
            "wfp": wfp if k == 0 else z((PIX, BD), np.float16),
            "wlp": wlp if k == NCORES - 1 else z((PIX, BD), np.float16),
            "wfc": wfc if k == 0 else z((CH, RC), np.float16),
            "wlc": wlc if k == NCORES - 1 else z((CH, RC * OUT), np.float16),
            "ident": ident,
        })
    return in_maps


def _assemble_m(results_a):
    mp_parts, mc_parts = [], []
    for k in range(NCORES):
        lo = 1 if k == 0 else 0
        hi = SLOTS - 1 if k == NCORES - 1 else SLOTS
        mp_parts.append(results_a[k]["mpsi"][lo:hi])
        mc_parts.append(results_a[k]["mchi"][lo:hi])
    mp_full = np.concatenate(mp_parts).reshape(NMID, B, BD, BD)
    mc_full = np.concatenate(mc_parts).reshape(NMID, B, RC, RC)
    return mp_full, mc_full


def _pack_psi(arr):
    """(nsites, l_or_r(64), 33, 64) site-major -> (ntiles, 128, 33*64)."""
    n = arr.shape[0]
    return np.ascontiguousarray(
        arr.reshape(n // 2, 2 * BD, BW * BD))


def _pack_chi(arr, ntiles):
    """(nsites, 32, 32, 32) -> (ntiles, 96, 1024) with zero pad."""
    n = arr.shape[0]
    out = np.zeros((ntiles, 3, RC, BSH * RC), arr.dtype)
    flat = arr.reshape(n, RC, BSH * RC)
    for s in range(n):
        out[s // 3, s % 3] = flat[s]
    return np.ascontiguousarray(out.reshape(ntiles, 3 * RC, BSH * RC))


def _prep_inputs_b(res_a, inputs):
    mp_full, mc_full = _assemble_m(res_a)   # (62,256,64,64), (62,256,32,32)
    v0p, v0c = res_a[0]["v0p"], res_a[0]["v0c"]
    wlast = res_a[NCORES - 1]["wlast"]
    tchi = res_a[NCORES - 1]["tchi"]
    ident = np.eye(RC, dtype=np.float32)
    # phi chain matrices (batch-independent): mid i uses phi_mid[i][:,:,i+1]
    phiw = np.stack([np.asarray(inputs["phi_mid"])[i][:, :, i + 1]
                     for i in range(NMID)]).astype(np.float32)
    phif0 = np.asarray(inputs["phi_first"])[:, 0].astype(np.float32)
    phil63 = np.asarray(inputs["phi_last"])[:, 63].astype(np.float32)
    in_maps_b = []
    for j in range(NCORES):
        sl = slice(BSH * j, BSH * (j + 1))
        # psi fwd: mids 0..31 as (site, l, b, r); phi rides as column 32
        pf = np.zeros((NPF, BD, BW, BD), np.float32)
        pf[:, :, 0:BSH, :] = mp_full[0:NPF, sl].transpose(0, 2, 1, 3)
        pf[:, :, BSH, :] = phiw[0:NPF]
        mpf = _pack_psi(pf.astype(np.float16))
        # psi bwd: mids 61..32 descending as (site, r, b, l); phi transposed
        pb = np.zeros((NPB, BD, BW, BD), np.float32)
        pb[:, :, 0:BSH, :] = (mp_full[NMID - 1:NMID - 1 - NPB:-1, sl]
                              .transpose(0, 3, 1, 2))
        pb[:, :, BSH, :] = phiw[NMID - 1:NMID - 1 - NPB:-1].transpose(0, 2, 1)
        mpb = _pack_psi(pb.astype(np.float16))
        # chi fwd: mids 0..30 as (site, l, b, r)
        mcf = _pack_chi(mc_full[0:NCF, sl].transpose(0, 2, 1, 3)
                        .astype(np.float16), CTF)
        # chi bwd: mids 61..31 descending as (site, r, b, l)
        mcb = _pack_chi(mc_full[NMID - 1:NMID - 1 - NCB:-1, sl]
                        .transpose(0, 3, 1, 2).astype(np.float16), CTB)
        tT = (tchi[sl].reshape(BSH, RC, OUT).transpose(1, 0, 2)
              .reshape(RC, BSH * OUT))
        NI = 2 * BW + BSH + BSH * OUT
        init = np.zeros((BD, NI), np.float16)
        init[0:BD, 0:BSH] = v0p[sl].T.astype(np.float16)
        init[0:BD, BSH] = phif0.astype(np.float16)
        init[0:BD, BW:BW + BSH] = wlast[sl].T.astype(np.float16)
        init[0:BD, BW + BSH] = phil63.astype(np.float16)
        init[0:RC, 2 * BW:2 * BW + BSH] = v0c[sl].T.astype(np.float16)
        init[0:RC, 2 * BW + BSH:] = tT.astype(np.float16)
        in_maps_b.append({
            "mpf": mpf, "mpb": mpb, "mcf": mcf, "mcb": mcb,
            "init": np.ascontiguousarray(init),
            "ident": ident,
        })
    return in_maps_b


def kernel(**inputs):
    core_ids = list(range(NCORES))
    if "nca" not in _cache:
        _cache["nca"] = build_launch_a()
        _cache["ncb"] = build_launch_b()
    nca, ncb = _cache["nca"], _cache["ncb"]

    LAST_RESULTS.clear()
    in_maps_a = _prep_inputs_a(inputs)
    LAST_INMAPS["a"] = in_maps_a
    bkr_a = run_bass_kernel_spmd(nca, in_maps_a, core_ids=core_ids)
    LAST_RESULTS.append(("launch_a", bkr_a))
    res_a = bkr_a.results

    in_maps_b = _prep_inputs_b(res_a, inputs)
    LAST_INMAPS["b"] = in_maps_b
    bkr_b = run_bass_kernel_spmd(ncb, in_maps_b, core_ids=core_ids)
    LAST_RESULTS.append(("launch_b", bkr_b))
    res_b = bkr_b.results

    out = np.empty((B, OUT), np.float32)
    for j in range(NCORES):
        out[BSH * j:BSH * (j + 1)] = res_b[j]["out"]
    return out


# revision 32
# speedup vs baseline: 1.0270x; 1.0270x over previous
"""Trainium2 Bass kernel for the CMPO3/GTN tensor-train contraction model.

Math (reference): three tensor-train chains over L=64 sites, each site
contracted with per-site input vectors derived from reductions of x:
  vpx[i,b,:] = mean_ch  x[b,i,:,:]   (PIX-dim vectors)
  vch[i,b,:] = mean_pix x[b,i,:,:]   (CH-dim vectors)
  psi chain (bond 64, phys PIX) -> scalar per batch
  chi chain (bond 32, phys CH)  -> (batch, 10)
  phi chain (bond 64, one-hot phys) -> global scalar
  out = chi_out * (psi_val * phi_val)[:, None]

Strategy (2 SPMD launches over 8 cores):
  Launch A (site/patch-sharded): each core owns 8 patches of x and the
    matching slices of psi_mid/chi_mid.  x is host-packed per site as
    [128 = (ch, p_hi), 8192 = (p_lo, b)] so BOTH data reductions run on
    the PE as matmuls against constant 0/1 selection matrices (E sums the
    ch lanes -> vpx, F sums over (ch,p_hi) with a 32-matmul accumulation
    group over p_lo -> vch); the 1/CH, 1/PIX mean scales are folded into
    E and F.  Cheap f16 PE transposes put vpxT/vchT into the
    p-on-partition layout the transfer-matrix matmuls need (psi weights
    are host-permuted to the matching (p_lo, p_hi) order).  The per-site
    transfer matrices
      M_s[b][l,r] = sum_p W_s[l,r,p] * u_s[b,p]
    go to DRAM as (site, b, l*r) f16.  PSUM->SBUF drains run on Act+DVE
    (the only engines allowed to read PSUM); DMAs are spread across
    SP/Act/Pool so the PE (~69us of matmul rows) stays the binding
    engine.  Boundary vectors are computed on the cores owning patch
    0 / 63.
  Launch B (batch-sharded): each core contracts the chains for its 32
    samples as four independent streams (psi fwd/bwd, chi fwd/bwd), each
    a sequence of per-batch stationary matvecs on the PE.  The
    batch-independent phi chain rides along as column 32 of the psi
    streams (its per-site 64x64 matrices are just one more stationary),
    so phi costs one extra matvec per site instead of a serial chain in
    launch A.  Stream state copies run on DVE/Act; M tile DMAs are
    spread across SP/Act/Pool.

All host-side work is layout glue only (transposes/slices/concats/dtype
casts plus folding the 1/CH, 1/PIX mean scales into constant selection
matrices).
"""

import sys

import numpy as np

if "/opt/trn_rl_repo" not in sys.path:
    sys.path.insert(0, "/opt/trn_rl_repo")

import concourse.bass as bass
import concourse.bacc as bacc
import concourse.mybir as mybir
import concourse.tile as tile
from concourse.bass_utils import run_bass_kernel_spmd

F32 = mybir.dt.float32
F16 = mybir.dt.float16
AX = mybir.AxisListType
ADD = mybir.AluOpType.add
MULT = mybir.AluOpType.mult

L, CH, PIX, PAT, RC, BD, OUT, B = 64, 16, 256, 64, 32, 64, 10, 256
NCORES = 8
SLOTS = 8          # patches per core in launch A
BSH = B // NCORES  # batch per core in launch B (32)
BW = BSH + 1       # psi stream width in launch B (batches + phi column)
NMID = L - 2       # 62 mid sites
NPF = 32           # psi fwd sites (mids 0..31)
NPB = 30           # psi bwd sites (mids 61..32)
NCF = 31           # chi fwd sites (mids 0..30)
NCB = 31           # chi bwd sites (mids 61..31)
PTF, PTB = NPF // 2, NPB // 2        # psi tiles per direction (2 sites/tile)
CTF, CTB = (NCF + 2) // 3, (NCB + 2) // 3  # chi tiles (3 sites/tile)
PGRP = 2           # psi tiles per DMA (after a small first group)
CGRP = 2           # chi tiles per DMA (after a small first group)

# p' permutation: the E-matmul/transpose pipeline yields vpxT rows ordered
# p' = p_lo*8 + p_hi where p = p_hi*32 + p_lo; psi weights (p-indexed) are
# host-permuted with PPERM so device contractions line up.
PPERM = np.array([(pp % 8) * 32 + pp // 8 for pp in range(PIX)])


# ---------------------------------------------------------------- launch A
def build_launch_a():
    nc = bacc.Bacc("TRN2", target_bir_lowering=False, debug=False,
                   num_devices=NCORES)
    # x per site: [128 = (ch, p_hi), (p_lo, b)] f16
    xe_in = nc.dram_tensor("xe", [SLOTS, 128, 64 * 128], F16, kind="ExternalInput").ap()
    # psi mids, p' permuted rows: (site, p', l*r)
    wpsi_in = nc.dram_tensor("wpsi", [SLOTS, PIX, BD * BD], F16, kind="ExternalInput").ap()
    # chi mids packed 3 slots/tile at 32-partition strides: (3, 96, rc*rc)
    wchi_in = nc.dram_tensor("wchi", [3, 96, RC * RC], F16, kind="ExternalInput").ap()
    # constant selection matrices (scales folded)
    emat_in = nc.dram_tensor("emat", [128, 8], F16, kind="ExternalInput").ap()
    fmat_in = nc.dram_tensor("fmat", [128, CH], F16, kind="ExternalInput").ap()
    wfp_in = nc.dram_tensor("wfp", [PIX, BD], F16, kind="ExternalInput").ap()
    wlp_in = nc.dram_tensor("wlp", [PIX, BD], F16, kind="ExternalInput").ap()
    wfc_in = nc.dram_tensor("wfc", [CH, RC], F16, kind="ExternalInput").ap()
    wlc_in = nc.dram_tensor("wlc", [CH, RC * OUT], F16, kind="ExternalInput").ap()
    ident_in = nc.dram_tensor("ident", [128, 128], F16, kind="ExternalInput").ap()

    mpsi_out = nc.dram_tensor("mpsi", [SLOTS, B, BD * BD], F16, kind="ExternalOutput").ap()
    mchi_out = nc.dram_tensor("mchi", [SLOTS, B, RC * RC], F16, kind="ExternalOutput").ap()
    v0p_out = nc.dram_tensor("v0p", [B, BD], F32, kind="ExternalOutput").ap()
    v0c_out = nc.dram_tensor("v0c", [B, RC], F32, kind="ExternalOutput").ap()
    wlast_out = nc.dram_tensor("wlast", [B, BD], F32, kind="ExternalOutput").ap()
    tchi_out = nc.dram_tensor("tchi", [B, RC * OUT], F32, kind="ExternalOutput").ap()

    with tile.TileContext(nc) as tc:
        with (
            tc.tile_pool(name="consts", bufs=1) as cpool,
            tc.tile_pool(name="xw", bufs=3) as xwpool,
            tc.tile_pool(name="vecs", bufs=2) as vpool,
            tc.tile_pool(name="mstage", bufs=2) as mpool,
            tc.tile_pool(name="small", bufs=2) as spool,
            tc.tile_pool(name="psmm", bufs=2, space="PSUM") as psmm,
            tc.tile_pool(name="pschi", bufs=1, space="PSUM") as pschi,
            tc.tile_pool(name="pssm", bufs=1, space="PSUM") as pssm,
        ):
            ident16 = cpool.tile([128, 128], F16, name="ident16")
            nc.sync.dma_start(out=ident16, in_=ident_in)
            emat = cpool.tile([128, 8], F16, name="emat")
            nc.sync.dma_start(out=emat, in_=emat_in)
            fmat = cpool.tile([128, CH], F16, name="fmat")
            nc.sync.dma_start(out=fmat, in_=fmat_in)
            # chi weights: slot s lives at rows 32*(s%3)..+16 of tile s//3
            wc_t = cpool.tile([96, 3, RC * RC], F16, name="wc_t")
            for t in range(3):
                nc.scalar.dma_start(out=wc_t[:, t, :], in_=wchi_in[t])
            # vchT accumulates per slot at rows 32*(s%3)..+16 of tile s//3
            vchT = cpool.tile([96, 3, B], F16, name="vchT")

            # boundary weights (p' permuted on host for the psi ones)
            wfp_t = cpool.tile([128, 2 * BD], F16, name="wfp_t")
            wlp_t = cpool.tile([128, 2 * BD], F16, name="wlp_t")
            for k in range(2):
                nc.scalar.dma_start(out=wfp_t[:, k * BD:(k + 1) * BD],
                                    in_=wfp_in[k * 128:(k + 1) * 128, :])
                nc.scalar.dma_start(out=wlp_t[:, k * BD:(k + 1) * BD],
                                    in_=wlp_in[k * 128:(k + 1) * 128, :])
            wfc_t = cpool.tile([CH, RC], F16, name="wfc_t")
            nc.scalar.dma_start(out=wfc_t, in_=wfc_in)
            wlc_t = cpool.tile([CH, RC * OUT], F16, name="wlc_t")
            nc.scalar.dma_start(out=wlc_t, in_=wlc_in)

            # boundary slots (0 on core 0, 7 on core 7) processed first to
            # shorten the tail; M writes for them land early too.
            ORDER = [0, SLOTS - 1] + list(range(1, SLOTS - 1))
            xe_tiles = {}
            wp_tiles = {}

            def fetch_xe(slot, early=False):
                t = xwpool.tile([128, 64 * 128], F16, name="xe_t",
                                tag="xe", bufs=3)
                engs = ([nc.gpsimd, nc.sync, nc.scalar, nc.gpsimd] if early
                        else [nc.sync, nc.scalar, nc.sync, nc.scalar])
                for q4 in range(4):
                    engs[q4].dma_start(
                        out=t[:, 2048 * q4:2048 * (q4 + 1)],
                        in_=xe_in[slot, :, 2048 * q4:2048 * (q4 + 1)])
                xe_tiles[slot] = t

            def fetch_wp(slot):
                t = xwpool.tile([128, 2, BD * BD], F16, name="wp",
                                tag="wp", bufs=3)
                nc.gpsimd.dma_start(out=t[:, 0, 0:2048],
                                    in_=wpsi_in[slot, 0:128, 0:2048])
                nc.gpsimd.dma_start(out=t[:, 0, 2048:4096],
                                    in_=wpsi_in[slot, 0:128, 2048:4096])
                nc.gpsimd.dma_start(out=t[:, 1, 0:2048],
                                    in_=wpsi_in[slot, 128:256, 0:2048])
                nc.sync.dma_start(out=t[:, 1, 2048:3072],
                                  in_=wpsi_in[slot, 128:256, 2048:3072])
                nc.gpsimd.dma_start(out=t[:, 1, 3072:4096],
                                    in_=wpsi_in[slot, 128:256, 3072:4096])
                wp_tiles[slot] = t

            fetch_xe(ORDER[0])
            fetch_xe(ORDER[1], early=True)
            fetch_wp(ORDER[0])
            fetch_wp(ORDER[1])
            stage1_out = {}

            def stage1(slot, oi):
                m3 = 32 * (slot % 3)   # vchT/wc partition base
                t3 = slot // 3
                if oi + 2 < SLOTS:
                    fetch_xe(ORDER[oi + 2])
                    fetch_wp(ORDER[oi + 2])
                xe_t = xe_tiles.pop(slot)

                # -------- data reductions on the PE
                # vpx: 64 E-matmuls -> cols 0:512 of a shared psum tile
                # (bank 0); vch F-accumulation lands in cols 512:544 (bank 1)
                pEF = psmm.tile([128, 1024], F32, name="pEF", tag="psEF",
                                bufs=1)
                pE = pEF[:, 0:512]
                pF = pEF[:, 512:512 + 2 * CH]
                for j in range(64):
                    bh = j % 2
                    pl = j // 2
                    sl = xe_t[:, 128 * j:128 * (j + 1)]
                    # one start/stop per psum bank: bank 0 holds all E
                    # outputs, bank 1 the F accumulators.  Disjoint writes
                    # after the start land on pending-zero bytes.
                    nc.tensor.matmul(pE[:, 256 * bh + 8 * pl:
                                        256 * bh + 8 * (pl + 1)], sl, emat,
                                     start=(j == 0), stop=(j == 63))
                    nc.tensor.matmul(pF[:, CH * bh:CH * (bh + 1)], sl, fmat,
                                     start=(j == 0), stop=(j == 63))
                v_t = vpool.tile([128, 2 * B], F16, name="V", tag="V")
                with nc.allow_low_precision(reason="vpx f16"):
                    nc.vector.tensor_copy(out=v_t, in_=pE)
                V = [v_t[:, 0:B], v_t[:, B:2 * B]]
                # transpose V chunks -> U[pc][128 p', 256 b]
                U = [vpool.tile([128, B], F16, name=f"U{pc}", tag=f"U{pc}")
                     for pc in range(2)]
                ptt = pssm.tile([128, 1024], F16, name="ptt", tag="pst",
                                bufs=1)
                for pc in range(2):
                    for bh in range(2):
                        tps = ptt[:, 128 * (2 * pc + bh):
                                  128 * (2 * pc + bh + 1)]
                        nc.tensor.transpose(
                            tps, V[bh][:, 128 * pc:128 * (pc + 1)], ident16)
                    nc.vector.tensor_copy(
                        out=U[pc], in_=ptt[:, 256 * pc:256 * (pc + 1)])
                # vch: copy + transpose into vchT rows m3..m3+16
                c_s = spool.tile([128, 2 * CH], F16, name="c_s", tag="c_s")
                with nc.allow_low_precision(reason="vch f16"):
                    nc.vector.tensor_copy(out=c_s, in_=pF)
                for bh in range(2):
                    tpc = ptt[0:CH, 512 + 128 * bh:512 + 128 * (bh + 1)]
                    nc.tensor.transpose(tpc, c_s[:, CH * bh:CH * (bh + 1)],
                                        ident16)
                nc.vector.tensor_copy(
                    out=vchT[m3:m3 + CH, t3, :], in_=ptt[0:CH, 512:768])
                stage1_out[slot] = U

            def stage2(slot, oi):
                m3 = 32 * (slot % 3)
                t3 = slot // 3
                U = stage1_out.pop(slot)
                # -------- psi mid transfer matrices
                wp = wp_tiles.pop(slot)
                mst = mpool.tile([128, 2, BD * BD], F16, name="mst", tag="mst")
                # 16 psum chunks per slot ([128,512], bufs=4) so the PE can
                # run several chunks ahead of the Act/DVE drains
                for bh in range(2):
                    for n in range(8):
                        ps = psmm.tile([128, 512], F32, name="ps",
                                       tag="ps_mm", bufs=3)
                        cs = slice(n * 512, (n + 1) * 512)
                        nc.tensor.matmul(
                            ps, U[0][:, bh * 128:(bh + 1) * 128],
                            wp[:, 0, cs], start=True, stop=False)
                        nc.tensor.matmul(
                            ps, U[1][:, bh * 128:(bh + 1) * 128],
                            wp[:, 1, cs], start=False, stop=True)
                        with nc.allow_low_precision(reason="m f16"):
                            if (n + bh) % 2:
                                nc.scalar.copy(
                                    mst[:, bh, cs], ps)
                            else:
                                nc.vector.tensor_copy(
                                    out=mst[:, bh, cs], in_=ps)
                    if oi == SLOTS - 1:
                        mqs = ([nc.gpsimd, nc.scalar] if bh == 0 else
                               [nc.sync, nc.gpsimd])
                    else:
                        mqs = ([nc.gpsimd, nc.gpsimd] if bh == 0 else
                               [nc.sync, nc.sync])
                    for hh in range(2):
                        mqs[hh].dma_start(
                            out=mpsi_out[slot, bh * 128:(bh + 1) * 128,
                                         2048 * hh:2048 * (hh + 1)],
                            in_=mst[:, bh, 2048 * hh:2048 * (hh + 1)])

                # -------- chi mid transfer matrices
                mstc = mpool.tile([128, 2, RC * RC], F16, name="mstc",
                                  tag="mstc")
                for bh in range(2):
                    psc = pschi.tile([128, 1024], F32, name="psc",
                                     tag="ps_chi", bufs=1)
                    for n in range(2):
                        nc.tensor.matmul(
                            psc[:, n * 512:(n + 1) * 512],
                            vchT[m3:m3 + CH, t3, bh * 128:(bh + 1) * 128],
                            wc_t[m3:m3 + CH, t3, n * 512:(n + 1) * 512],
                            start=True, stop=True)
                    with nc.allow_low_precision(reason="mc f16"):
                        if bh:
                            nc.scalar.copy(mstc[:, bh, :], psc)
                        else:
                            nc.vector.tensor_copy(out=mstc[:, bh, :], in_=psc)
                nc.sync.dma_start(out=mchi_out[slot].rearrange(
                    "(c b) f -> b c f", c=2), in_=mstc)

                # -------- boundary contractions (host keeps core0/core7 only)
                if slot in (0, SLOTS - 1):
                    # base-0 copy of this slot's vchT for the boundary matmuls
                    vch0 = spool.tile([CH, B], F16, name="vch0", tag="vch0")
                    nc.gpsimd.tensor_copy(out=vch0,
                                          in_=vchT[m3:m3 + CH, t3, :])
                    wpt = wfp_t if slot == 0 else wlp_t
                    pout = v0p_out if slot == 0 else wlast_out
                    cout = v0c_out if slot == 0 else tchi_out
                    wct_b = wfc_t if slot == 0 else wlc_t
                    cw = RC if slot == 0 else RC * OUT
                    for bh in range(2):
                        psbt = psmm.tile([128, 512], F32, name="ps",
                                         tag="ps_mm", bufs=3)
                        psb = psbt[:, 0:BD]
                        for k in range(2):
                            nc.tensor.matmul(psb,
                                             U[k][:, bh * 128:(bh + 1) * 128],
                                             wpt[:, k * BD:(k + 1) * BD],
                                             start=(k == 0), stop=(k == 1))
                        v0s = spool.tile([128, BD], F32, name="v0s", tag="bnd")
                        nc.vector.tensor_copy(out=v0s, in_=psb)
                        nc.gpsimd.dma_start(out=pout[bh * 128:(bh + 1) * 128, :],
                                            in_=v0s)
                        psct = psmm.tile([128, 512], F32, name="ps",
                                         tag="ps_mm", bufs=3)
                        psc0 = psct[:, 0:cw]
                        nc.tensor.matmul(psc0, vch0[:, bh * 128:(bh + 1) * 128],
                                         wct_b, start=True, stop=True)
                        v0cs = spool.tile([128, cw], F32, name="v0cs",
                                          tag="bndc")
                        nc.vector.tensor_copy(out=v0cs, in_=psc0)
                        nc.gpsimd.dma_start(out=cout[bh * 128:(bh + 1) * 128, :],
                                            in_=v0cs)

            stage1(ORDER[0], 0)
            for oi in range(SLOTS):
                if oi + 1 < SLOTS:
                    stage1(ORDER[oi + 1], oi + 1)
                stage2(ORDER[oi], oi)
    nc.finalize()
    return nc


# ---------------------------------------------------------------- launch B
def build_launch_b():
    """Batch-sharded chains as four per-batch stationary-matvec streams.

    Each stream holds its state as an f16 [bond, width] SBUF tile whose
    partition base cycles with the site index (psi: 0/64; chi: 0/32/64),
    matching where the host packed that site's stationary matrix in its
    DMA tile (matmul requires lhsT/rhs/psum bases to agree and be in
    {0,32,64}).  A site = `width` single-column matmuls (one per batch,
    PSUM column out) + one PSUM->SBUF f16 state copy (DVE/Act).  The psi
    streams carry the batch-independent phi chain as column 32.  The chi
    bwd stream carries a matrix state (32l x 10o per batch).  Finals: the
    psi/phi dot via a ones-matmul partition reduce; chi fwd/bwd per-batch
    dots to [10, 32b], transposed and scaled by psi*phi on the DVE.
    """
    nc = bacc.Bacc("TRN2", target_bir_lowering=False, debug=False,
                   num_devices=NCORES)
    mpf_in = nc.dram_tensor("mpf", [PTF, 128, BW * BD], F16, kind="ExternalInput").ap()
    mpb_in = nc.dram_tensor("mpb", [PTB, 128, BW * BD], F16, kind="ExternalInput").ap()
    mcf_in = nc.dram_tensor("mcf", [CTF, 96, BSH * RC], F16, kind="ExternalInput").ap()
    mcb_in = nc.dram_tensor("mcb", [CTB, 96, BSH * RC], F16, kind="ExternalInput").ap()
    # packed initial states: cols 0:33 v0pT|phi0, 33:66 wlT|phiL (rows 0:64);
    # cols 66:98 v0cT (rows 0:32), cols 98:418 tT (rows 0:32)
    NI = 2 * BW + BSH + BSH * OUT
    init_in = nc.dram_tensor("init", [BD, NI], F16, kind="ExternalInput").ap()
    ident_in = nc.dram_tensor("ident", [RC, RC], F32, kind="ExternalInput").ap()

    out_out = nc.dram_tensor("out", [BSH, OUT], F32, kind="ExternalOutput").ap()

    with tile.TileContext(nc) as tc:
        with (
            tc.tile_pool(name="consts", bufs=1) as cpool,
            tc.tile_pool(name="mload", bufs=2) as mpool,
            tc.tile_pool(name="states", bufs=2) as spool,
            tc.tile_pool(name="psA", bufs=1, space="PSUM") as psA,
            tc.tile_pool(name="psB", bufs=1, space="PSUM") as psB,
        ):
            ident_t = cpool.tile([RC, RC], F32, name="ident_t")
            nc.gpsimd.dma_start(out=ident_t, in_=ident_in)
            ones32 = cpool.tile([128, 1], F32, name="ones32")
            nc.vector.memset(ones32, 1.0)
            onesw = cpool.tile([128, BSH], F32, name="onesw")
            nc.vector.memset(onesw, 1.0)

            # stream initial states, one packed DMA
            init_t = cpool.tile([BD, NI], F16, name="init_t")
            nc.sync.dma_start(out=init_t, in_=init_in)
            stf = init_t[0:BD, 0:BW]
            stb = init_t[0:BD, BW:2 * BW]
            stc = init_t[0:RC, 2 * BW:2 * BW + BSH]
            stg = init_t[0:RC, 2 * BW + BSH:NI]

            # group DMA tiles for the four streams: one shared weighted
            # rotation over the three DMA-capable queues (DVE can't DMA);
            # Act gets a lighter share since it also drains chi-bwd states
            ROT = [nc.sync, nc.gpsimd, nc.scalar, nc.sync, nc.gpsimd,
                   nc.scalar, nc.sync, nc.gpsimd]
            _gctr = {"n": 0}

            def load_group(tag, dram, t0, ntiles, width):
                gt = mpool.tile([dram.shape[1], ntiles, width], F16,
                                name=f"g_{tag}", tag=f"g_{tag}", bufs=3)
                q = ROT[_gctr["n"] % len(ROT)]
                _gctr["n"] += 1
                q.dma_start(
                    out=gt, in_=dram[t0:t0 + ntiles].rearrange("t p f -> p t f"))
                return gt

            # Each stream is a generator yielding once per site so the four
            # chains can be emitted interleaved (round-robin): the PE executes
            # its queue in program order, so sequential emission would
            # serialize the streams' latencies.
            def stream_steps(tag, dram, nsites, state, ps_pool,
                             bond, per_tile, grp, owidth, ncols, result,
                             copy_engines):
                gt = None
                ntiles_tot = (nsites + per_tile - 1) // per_tile
                # group boundaries: first group small (2) so the stream can
                # start as soon as possible; then groups of `grp`
                bounds = [0, min(1, ntiles_tot)]
                while bounds[-1] < ntiles_tot:
                    bounds.append(min(bounds[-1] + grp, ntiles_tot))
                tile2group = {}
                for gi in range(len(bounds) - 1):
                    for t in range(bounds[gi], bounds[gi + 1]):
                        tile2group[t] = (gi, bounds[gi], t - bounds[gi])
                for s in range(nsites):
                    t_idx, off = divmod(s, per_tile)
                    gi, g0, g_off = tile2group[t_idx]
                    if t_idx == g0 and off == 0:
                        n = bounds[gi + 1] - g0
                        gt = load_group(tag, dram, g0, n, ncols * bond)
                    base = bond * off
                    nbase = bond * ((s + 1) % per_tile)
                    ps = ps_pool.tile([128, ncols * owidth], F32,
                                      name=f"ps_{tag}", tag=f"ps_{tag}",
                                      bufs=1)
                    for b in range(ncols):
                        nc.tensor.matmul(
                            ps[nbase:nbase + bond, owidth * b:owidth * (b + 1)],
                            gt[base:base + bond, g_off,
                               bond * b:bond * (b + 1)],
                            state[base:base + bond,
                                  owidth * b:owidth * (b + 1)],
                            start=True, stop=True)
                    state = spool.tile([128, ncols * owidth], F16,
                                       name=f"st_{tag}", tag=tag)
                    ceng = copy_engines[s % len(copy_engines)]
                    with nc.allow_low_precision(reason="f16 chain state"):
                        if ceng is nc.scalar:
                            ceng.copy(state[nbase:nbase + bond, :],
                                      ps[nbase:nbase + bond, :])
                        else:
                            ceng.tensor_copy(out=state[nbase:nbase + bond, :],
                                             in_=ps[nbase:nbase + bond, :])
                    yield
                result.append(state)

            res_f, res_b, res_c, res_g = [], [], [], []
            gens = [
                stream_steps("stf", mpf_in, NPF, stf, psA,
                             BD, 2, PGRP, 1, BW, res_f, [nc.vector]),
                stream_steps("stb", mpb_in, NPB, stb, psA,
                             BD, 2, PGRP, 1, BW, res_b, [nc.vector]),
                stream_steps("stc", mcf_in, NCF, stc, psB,
                             RC, 3, CGRP, 1, BSH, res_c, [nc.vector]),
                stream_steps("stg", mcb_in, NCB, stg, psB,
                             RC, 3, CGRP, OUT, BSH, res_g,
                             [nc.scalar, nc.vector]),
            ]
            live = list(gens)
            while live:
                for g in list(live):
                    try:
                        next(g)
                    except StopIteration:
                        live.remove(g)
            stf, stb, stc, stg = res_f[0], res_b[0], res_c[0], res_g[0]

            fb_f = BD * (NPF % 2)   # 0
            fb_b = BD * (NPB % 2)   # 0
            fb_c = RC * (NCF % 3)   # 32
            fb_g = RC * (NCB % 3)   # 32

            # psi_val[b] = sum_l stf[l,b]*stb[l,b]; col 32 gives phi_val.
            # f32 throughout: the products are ~1e-8 and underflow in f16.
            prod = spool.tile([128, BW], F32, name="prod", tag="prod")
            nc.vector.tensor_tensor(out=prod[fb_f:fb_f + BD, :],
                                    in0=stf[fb_f:fb_f + BD, :],
                                    in1=stb[fb_b:fb_b + BD, :],
                                    op=MULT)
            ppv = psA.tile([BW, 1], F32, name="ppv", tag="ppv", bufs=1)
            nc.tensor.matmul(ppv, prod[fb_f:fb_f + BD, :],
                             ones32[fb_f:fb_f + BD, :], start=True, stop=True)
            psiphi = spool.tile([BW, 1], F32, name="psiphi", tag="fin")
            nc.vector.tensor_copy(out=psiphi, in_=ppv)
            # replicate phival (row 32) across the 32 batch partitions via a
            # ones-row matmul (bases must agree at 32), then fold into psival
            prep = psA.tile([BSH, 1], F32, name="prep", tag="prep", bufs=1)
            nc.tensor.matmul(prep, onesw[32:33, :], psiphi[BSH:BW, 0:1],
                             start=True, stop=True)
            scal = spool.tile([BSH, 1], F32, name="scal", tag="fin2c")
            nc.vector.tensor_tensor(out=scal, in0=psiphi[0:BSH, :],
                                    in1=prep, op=MULT)

            # chi_out[o,b] = sum_l stg[l, b*OUT+o] * stc[l, b]
            pcf = psB.tile([OUT, BSH], F32, name="pcf", tag="pcf", bufs=1)
            for b in range(BSH):
                nc.tensor.matmul(pcf[:, b:b + 1],
                                 stg[fb_g:fb_g + RC, OUT * b:OUT * (b + 1)],
                                 stc[fb_c:fb_c + RC, b:b + 1],
                                 start=True, stop=True)
            chifs = spool.tile([OUT, BSH], F32, name="chifs", tag="fin2")
            nc.vector.tensor_copy(out=chifs, in_=pcf)
            pt = psA.tile([BSH, OUT], F32, name="pt", tag="pt", bufs=1)
            nc.tensor.transpose(pt, chifs, ident_t[0:OUT, 0:OUT])
            res = spool.tile([BSH, OUT], F32, name="res", tag="fin3")
            nc.vector.tensor_scalar_mul(out=res, in0=pt, scalar1=scal)
            nc.sync.dma_start(out=out_out, in_=res)
    nc.finalize()
    return nc


# ------------------------------------------------------------- host glue
_cache = {}
LAST_RESULTS = []  # [(label, BassKernelResults)] from the most recent kernel()
LAST_INMAPS = {}   # {"a": in_maps_a, "b": in_maps_b} from the most recent kernel()


def _prep_inputs_a(inputs):
    # f16 upload of x: the on-device reductions accumulate in f32; the
    # 0.05% per-element cast error is far below the f16 weight error.
    x = np.asarray(inputs["x"], dtype=np.float32)
    # [site, (ch, p_hi), (p_lo, b)]: x[b, site, p, c] with p = ph*32 + pl
    xe = np.ascontiguousarray(
        x.transpose(1, 3, 2, 0)                       # (site, c, p, b)
        .reshape(PAT, CH, 8, 32, B)                   # (site, c, ph, pl, b)
        .reshape(PAT, 128, 32 * B).astype(np.float16))

    # psi_mid (62,l,r,p) -> (62, p', l*r), rows p'-permuted (no /CH: E has it)
    pm = inputs["psi_mid"].astype(np.float32)
    wpsi = np.ascontiguousarray(
        pm.transpose(0, 3, 1, 2).reshape(NMID, PIX, BD * BD)[:, PPERM, :])
    # chi_mid (62,l,r,ch) -> (62, ch, rc*rc) (no /PIX: F has it)
    cm = inputs["chi_mid"].astype(np.float32)
    wchi = np.ascontiguousarray(
        cm.transpose(0, 3, 1, 2).reshape(NMID, CH, RC * RC))

    wfp = np.ascontiguousarray(
        inputs["psi_first"].T.astype(np.float32)[PPERM, :]).astype(np.float16)
    wlp = np.ascontiguousarray(
        inputs["psi_last"].T.astype(np.float32)[PPERM, :]).astype(np.float16)
    wfc = np.ascontiguousarray(
        inputs["chi_first"].T.astype(np.float32)).astype(np.float16)
    wlc = np.ascontiguousarray(
        inputs["chi_last"].astype(np.float32).transpose(1, 0, 2)
        .reshape(CH, RC * OUT)).astype(np.float16)

    ident = np.eye(128, dtype=np.float16)

    # selection matrices: rows (c, ph); E sums over c (vpx = mean_c),
    # F sums over ph (+ the p_lo accumulation group) for vch = mean_p.
    emat = np.zeros((128, 8), np.float16)
    fmat = np.zeros((128, CH), np.float16)
    for q in range(128):
        c, ph = q // 8, q % 8
        emat[q, ph] = 1.0 / CH
        fmat[q, c] = 1.0 / PIX

    zero_pw = np.zeros_like(wpsi[0])
    zero_cw = np.zeros_like(wchi[0])
    in_maps = []
    for k in range(NCORES):
        # slot j of core k handles patch 8k+j; mid site s uses weight s-1
        wp_slots = np.stack([
            wpsi[8 * k + j - 1] if 1 <= 8 * k + j <= NMID else zero_pw
            for j in range(SLOTS)]).astype(np.float16)
        wc_slots = np.stack([
            wchi[8 * k + j - 1] if 1 <= 8 * k + j <= NMID else zero_cw
            for j in range(SLOTS)]).astype(np.float16)
        # pack chi weights: tile t rows 32*m..+16 = slot 3t+m
        wc_packed = np.zeros((3, 96, RC * RC), np.float16)
        for j in range(SLOTS):
            wc_packed[j // 3, 32 * (j % 3):32 * (j % 3) + CH] = wc_slots[j]
        z = np.zeros
        in_maps.append({
            "xe": np.ascontiguousarray(xe[8 * k:8 * (k + 1)]),
            "wpsi": np.ascontiguousarray(wp_slots),
            "wchi": wc_packed,
            "emat": emat,
            "fmat": fmat,
            "wfp": wfp if k == 0 else z((PIX, BD), np.float16),
            "wlp": wlp if k == NCORES - 1 else z((PIX, BD), np.float16),
            "wfc": wfc if k == 0 else z((CH, RC), np.float16),
            "wlc": wlc if k == NCORES - 1 else z((CH, RC * OUT), np.float16),
            "ident": ident,
        })
    return in_maps


def _assemble_m(results_a):
    mp_parts, mc_parts = [], []
    for k in range(NCORES):
        lo = 1 if k == 0 else 0
        hi = SLOTS - 1 if k == NCORES - 1 else SLOTS
        mp_parts.append(results_a[k]["mpsi"][lo:hi])
        mc_parts.append(results_a[k]["mchi"][lo:hi])
    mp_full = np.concatenate(mp_parts).reshape(NMID, B, BD, BD)
    mc_full = np.concatenate(mc_parts).reshape(NMID, B, RC, RC)
    return mp_full, mc_full


def _pack_psi(arr):
    """(nsites, l_or_r(64), 33, 64) site-major -> (ntiles, 128, 33*64)."""
    n = arr.shape[0]
    return np.ascontiguousarray(
        arr.reshape(n // 2, 2 * BD, BW * BD))


def _pack_chi(arr, ntiles):
    """(nsites, 32, 32, 32) -> (ntiles, 96, 1024) with zero pad."""
    n = arr.shape[0]
    out = np.zeros((ntiles, 3, RC, BSH * RC), arr.dtype)
    flat = arr.reshape(n, RC, BSH * RC)
    for s in range(n):
        out[s // 3, s % 3] = flat[s]
    return np.ascontiguousarray(out.reshape(ntiles, 3 * RC, BSH * RC))


def _prep_inputs_b(res_a, inputs):
    mp_full, mc_full = _assemble_m(res_a)   # (62,256,64,64), (62,256,32,32)
    v0p, v0c = res_a[0]["v0p"], res_a[0]["v0c"]
    wlast = res_a[NCORES - 1]["wlast"]
    tchi = res_a[NCORES - 1]["tchi"]
    ident = np.eye(RC, dtype=np.float32)
    # phi chain matrices (batch-independent): mid i uses phi_mid[i][:,:,i+1]
    phiw = np.stack([np.asarray(inputs["phi_mid"])[i][:, :, i + 1]
                     for i in range(NMID)]).astype(np.float32)
    phif0 = np.asarray(inputs["phi_first"])[:, 0].astype(np.float32)
    phil63 = np.asarray(inputs["phi_last"])[:, 63].astype(np.float32)
    in_maps_b = []
    for j in range(NCORES):
        sl = slice(BSH * j, BSH * (j + 1))
        # psi fwd: mids 0..31 as (site, l, b, r); phi rides as column 32
        pf = np.zeros((NPF, BD, BW, BD), np.float32)
        pf[:, :, 0:BSH, :] = mp_full[0:NPF, sl].transpose(0, 2, 1, 3)
        pf[:, :, BSH, :] = phiw[0:NPF]
        mpf = _pack_psi(pf.astype(np.float16))
        # psi bwd: mids 61..32 descending as (site, r, b, l); phi transposed
        pb = np.zeros((NPB, BD, BW, BD), np.float32)
        pb[:, :, 0:BSH, :] = (mp_full[NMID - 1:NMID - 1 - NPB:-1, sl]
                              .transpose(0, 3, 1, 2))
        pb[:, :, BSH, :] = phiw[NMID - 1:NMID - 1 - NPB:-1].transpose(0, 2, 1)
        mpb = _pack_psi(pb.astype(np.float16))
        # chi fwd: mids 0..30 as (site, l, b, r)
        mcf = _pack_chi(mc_full[0:NCF, sl].transpose(0, 2, 1, 3)
                        .astype(np.float16), CTF)
        # chi bwd: mids 61..31 descending as (site, r, b, l)
        mcb = _pack_chi(mc_full[NMID - 1:NMID - 1 - NCB:-1, sl]
                        .transpose(0, 3, 1, 2).astype(np.float16), CTB)
        tT = (tchi[sl].reshape(BSH, RC, OUT).transpose(1, 0, 2)
              .reshape(RC, BSH * OUT))
        NI = 2 * BW + BSH + BSH * OUT
        init = np.zeros((BD, NI), np.float16)
        init[0:BD, 0:BSH] = v0p[sl].T.astype(np.float16)
        init[0:BD, BSH] = phif0.astype(np.float16)
        init[0:BD, BW:BW + BSH] = wlast[sl].T.astype(np.float16)
        init[0:BD, BW + BSH] = phil63.astype(np.float16)
        init[0:RC, 2 * BW:2 * BW + BSH] = v0c[sl].T.astype(np.float16)
        init[0:RC, 2 * BW + BSH:] = tT.astype(np.float16)
        in_maps_b.append({
            "mpf": mpf, "mpb": mpb, "mcf": mcf, "mcb": mcb,
            "init": np.ascontiguousarray(init),
            "ident": ident,
        })
    return in_maps_b


def kernel(**inputs):
    core_ids = list(range(NCORES))
    if "nca" not in _cache:
        _cache["nca"] = build_launch_a()
        _cache["ncb"] = build_launch_b()
    nca, ncb = _cache["nca"], _cache["ncb"]

    LAST_RESULTS.clear()
    in_maps_a = _prep_inputs_a(inputs)
    LAST_INMAPS["a"] = in_maps_a
    bkr_a = run_bass_kernel_spmd(nca, in_maps_a, core_ids=core_ids)
    LAST_RESULTS.append(("launch_a", bkr_a))
    res_a = bkr_a.results

    in_maps_b = _prep_inputs_b(res_a, inputs)
    LAST_INMAPS["b"] = in_maps_b
    bkr_b = run_bass_kernel_spmd(ncb, in_maps_b, core_ids=core_ids)
    LAST_RESULTS.append(("launch_b", bkr_b))
    res_b = bkr_b.results

    out = np.empty((B, OUT), np.float32)
    for j in range(NCORES):
        out[BSH * j:BSH * (j + 1)] = res_b[j]["out"]
    return out


# revision 33
# speedup vs baseline: 1.0287x; 1.0017x over previous
"""Trainium2 Bass kernel for the CMPO3/GTN tensor-train contraction model.

Math (reference): three tensor-train chains over L=64 sites, each site
contracted with per-site input vectors derived from reductions of x:
  vpx[i,b,:] = mean_ch  x[b,i,:,:]   (PIX-dim vectors)
  vch[i,b,:] = mean_pix x[b,i,:,:]   (CH-dim vectors)
  psi chain (bond 64, phys PIX) -> scalar per batch
  chi chain (bond 32, phys CH)  -> (batch, 10)
  phi chain (bond 64, one-hot phys) -> global scalar
  out = chi_out * (psi_val * phi_val)[:, None]

Strategy (2 SPMD launches over 8 cores):
  Launch A (site/patch-sharded): each core owns 8 patches of x and the
    matching slices of psi_mid/chi_mid.  x is host-packed per site as
    [128 = (ch, p_hi), 8192 = (p_lo, b)] so BOTH data reductions run on
    the PE as matmuls against constant 0/1 selection matrices (E sums the
    ch lanes -> vpx, F sums over (ch,p_hi) with a 32-matmul accumulation
    group over p_lo -> vch); the 1/CH, 1/PIX mean scales are folded into
    E and F.  Cheap f16 PE transposes put vpxT/vchT into the
    p-on-partition layout the transfer-matrix matmuls need (psi weights
    are host-permuted to the matching (p_lo, p_hi) order).  The per-site
    transfer matrices
      M_s[b][l,r] = sum_p W_s[l,r,p] * u_s[b,p]
    go to DRAM as (site, b, l*r) f16.  PSUM->SBUF drains run on Act+DVE
    (the only engines allowed to read PSUM); DMAs are spread across
    SP/Act/Pool so the PE (~69us of matmul rows) stays the binding
    engine.  Boundary vectors are computed on the cores owning patch
    0 / 63.
  Launch B (batch-sharded): each core contracts the chains for its 32
    samples as four independent streams (psi fwd/bwd, chi fwd/bwd), each
    a sequence of per-batch stationary matvecs on the PE.  The
    batch-independent phi chain rides along as column 32 of the psi
    streams (its per-site 64x64 matrices are just one more stationary),
    so phi costs one extra matvec per site instead of a serial chain in
    launch A.  Stream state copies run on DVE/Act; M tile DMAs are
    spread across SP/Act/Pool.

All host-side work is layout glue only (transposes/slices/concats/dtype
casts plus folding the 1/CH, 1/PIX mean scales into constant selection
matrices).
"""

import sys

import numpy as np

if "/opt/trn_rl_repo" not in sys.path:
    sys.path.insert(0, "/opt/trn_rl_repo")

import concourse.bass as bass
import concourse.bacc as bacc
import concourse.mybir as mybir
import concourse.tile as tile
from concourse.bass_utils import run_bass_kernel_spmd

F32 = mybir.dt.float32
F16 = mybir.dt.float16
AX = mybir.AxisListType
ADD = mybir.AluOpType.add
MULT = mybir.AluOpType.mult

L, CH, PIX, PAT, RC, BD, OUT, B = 64, 16, 256, 64, 32, 64, 10, 256
NCORES = 8
SLOTS = 8          # patches per core in launch A
BSH = B // NCORES  # batch per core in launch B (32)
BW = BSH + 1       # psi stream width in launch B (batches + phi column)
NMID = L - 2       # 62 mid sites
NPF = 32           # psi fwd sites (mids 0..31)
NPB = 30           # psi bwd sites (mids 61..32)
NCF = 31           # chi fwd sites (mids 0..30)
NCB = 31           # chi bwd sites (mids 61..31)
PTF, PTB = NPF // 2, NPB // 2        # psi tiles per direction (2 sites/tile)
CTF, CTB = (NCF + 2) // 3, (NCB + 2) // 3  # chi tiles (3 sites/tile)
PGRP = 2           # psi tiles per DMA (after a small first group)
CGRP = 2           # chi tiles per DMA (after a small first group)

# p' permutation: the E-matmul/transpose pipeline yields vpxT rows ordered
# p' = p_lo*8 + p_hi where p = p_hi*32 + p_lo; psi weights (p-indexed) are
# host-permuted with PPERM so device contractions line up.
PPERM = np.array([(pp % 8) * 32 + pp // 8 for pp in range(PIX)])


# ---------------------------------------------------------------- launch A
def build_launch_a():
    nc = bacc.Bacc("TRN2", target_bir_lowering=False, debug=False,
                   num_devices=NCORES)
    # x per site: [128 = (ch, p_hi), (p_lo, b)] f16
    xe_in = nc.dram_tensor("xe", [SLOTS, 128, 64 * 128], F16, kind="ExternalInput").ap()
    # psi mids, p' permuted rows: (site, p', l*r)
    wpsi_in = nc.dram_tensor("wpsi", [SLOTS, PIX, BD * BD], F16, kind="ExternalInput").ap()
    # chi mids packed 3 slots/tile at 32-partition strides: (3, 96, rc*rc)
    wchi_in = nc.dram_tensor("wchi", [3, 96, RC * RC], F16, kind="ExternalInput").ap()
    # constant selection matrices (scales folded)
    emat_in = nc.dram_tensor("emat", [128, 8], F16, kind="ExternalInput").ap()
    fmat_in = nc.dram_tensor("fmat", [128, CH], F16, kind="ExternalInput").ap()
    wfp_in = nc.dram_tensor("wfp", [PIX, BD], F16, kind="ExternalInput").ap()
    wlp_in = nc.dram_tensor("wlp", [PIX, BD], F16, kind="ExternalInput").ap()
    wfc_in = nc.dram_tensor("wfc", [CH, RC], F16, kind="ExternalInput").ap()
    wlc_in = nc.dram_tensor("wlc", [CH, RC * OUT], F16, kind="ExternalInput").ap()
    ident_in = nc.dram_tensor("ident", [128, 128], F16, kind="ExternalInput").ap()

    mpsi_out = nc.dram_tensor("mpsi", [SLOTS, B, BD * BD], F16, kind="ExternalOutput").ap()
    mchi_out = nc.dram_tensor("mchi", [SLOTS, B, RC * RC], F16, kind="ExternalOutput").ap()
    v0p_out = nc.dram_tensor("v0p", [B, BD], F32, kind="ExternalOutput").ap()
    v0c_out = nc.dram_tensor("v0c", [B, RC], F32, kind="ExternalOutput").ap()
    wlast_out = nc.dram_tensor("wlast", [B, BD], F32, kind="ExternalOutput").ap()
    tchi_out = nc.dram_tensor("tchi", [B, RC * OUT], F32, kind="ExternalOutput").ap()

    with tile.TileContext(nc) as tc:
        with (
            tc.tile_pool(name="consts", bufs=1) as cpool,
            tc.tile_pool(name="xw", bufs=3) as xwpool,
            tc.tile_pool(name="vecs", bufs=2) as vpool,
            tc.tile_pool(name="mstage", bufs=2) as mpool,
            tc.tile_pool(name="small", bufs=2) as spool,
            tc.tile_pool(name="psmm", bufs=2, space="PSUM") as psmm,
            tc.tile_pool(name="pschi", bufs=1, space="PSUM") as pschi,
            tc.tile_pool(name="pssm", bufs=1, space="PSUM") as pssm,
        ):
            ident16 = cpool.tile([128, 128], F16, name="ident16")
            nc.sync.dma_start(out=ident16, in_=ident_in)
            emat = cpool.tile([128, 8], F16, name="emat")
            nc.sync.dma_start(out=emat, in_=emat_in)
            fmat = cpool.tile([128, CH], F16, name="fmat")
            nc.sync.dma_start(out=fmat, in_=fmat_in)
            # chi weights: slot s lives at rows 32*(s%3)..+16 of tile s//3
            wc_t = cpool.tile([96, 3, RC * RC], F16, name="wc_t")
            for t in range(3):
                nc.scalar.dma_start(out=wc_t[:, t, :], in_=wchi_in[t])
            # vchT accumulates per slot at rows 32*(s%3)..+16 of tile s//3
            vchT = cpool.tile([96, 3, B], F16, name="vchT")

            # boundary weights (p' permuted on host for the psi ones)
            wfp_t = cpool.tile([128, 2 * BD], F16, name="wfp_t")
            wlp_t = cpool.tile([128, 2 * BD], F16, name="wlp_t")
            for k in range(2):
                nc.scalar.dma_start(out=wfp_t[:, k * BD:(k + 1) * BD],
                                    in_=wfp_in[k * 128:(k + 1) * 128, :])
                nc.scalar.dma_start(out=wlp_t[:, k * BD:(k + 1) * BD],
                                    in_=wlp_in[k * 128:(k + 1) * 128, :])
            wfc_t = cpool.tile([CH, RC], F16, name="wfc_t")
            nc.scalar.dma_start(out=wfc_t, in_=wfc_in)
            wlc_t = cpool.tile([CH, RC * OUT], F16, name="wlc_t")
            nc.scalar.dma_start(out=wlc_t, in_=wlc_in)

            # boundary slots (0 on core 0, 7 on core 7) processed first to
            # shorten the tail; M writes for them land early too.
            ORDER = [0, SLOTS - 1] + list(range(1, SLOTS - 1))
            xe_tiles = {}
            wp_tiles = {}

            def fetch_xe(slot, early=False):
                t = xwpool.tile([128, 64 * 128], F16, name="xe_t",
                                tag="xe", bufs=4)
                engs = ([nc.gpsimd, nc.sync, nc.scalar, nc.gpsimd] if early
                        else [nc.sync, nc.scalar, nc.sync, nc.scalar])
                for q4 in range(4):
                    engs[q4].dma_start(
                        out=t[:, 2048 * q4:2048 * (q4 + 1)],
                        in_=xe_in[slot, :, 2048 * q4:2048 * (q4 + 1)])
                xe_tiles[slot] = t

            def fetch_wp(slot):
                t = xwpool.tile([128, 2, BD * BD], F16, name="wp",
                                tag="wp", bufs=3)
                nc.gpsimd.dma_start(out=t[:, 0, 0:2048],
                                    in_=wpsi_in[slot, 0:128, 0:2048])
                nc.gpsimd.dma_start(out=t[:, 0, 2048:4096],
                                    in_=wpsi_in[slot, 0:128, 2048:4096])
                nc.gpsimd.dma_start(out=t[:, 1, 0:2048],
                                    in_=wpsi_in[slot, 128:256, 0:2048])
                nc.sync.dma_start(out=t[:, 1, 2048:3072],
                                  in_=wpsi_in[slot, 128:256, 2048:3072])
                nc.gpsimd.dma_start(out=t[:, 1, 3072:4096],
                                    in_=wpsi_in[slot, 128:256, 3072:4096])
                wp_tiles[slot] = t

            fetch_xe(ORDER[0])
            fetch_xe(ORDER[1], early=True)
            fetch_wp(ORDER[0])
            fetch_wp(ORDER[1])
            stage1_out = {}

            def stage1(slot, oi):
                m3 = 32 * (slot % 3)   # vchT/wc partition base
                t3 = slot // 3
                if oi + 2 < SLOTS:
                    fetch_xe(ORDER[oi + 2])
                    fetch_wp(ORDER[oi + 2])
                xe_t = xe_tiles.pop(slot)

                # -------- data reductions on the PE
                # vpx: 64 E-matmuls -> cols 0:512 of a shared psum tile
                # (bank 0); vch F-accumulation lands in cols 512:544 (bank 1)
                pEF = psmm.tile([128, 1024], F32, name="pEF", tag="psEF",
                                bufs=1)
                pE = pEF[:, 0:512]
                pF = pEF[:, 512:512 + 2 * CH]
                for j in range(64):
                    bh = j % 2
                    pl = j // 2
                    sl = xe_t[:, 128 * j:128 * (j + 1)]
                    # one start/stop per psum bank: bank 0 holds all E
                    # outputs, bank 1 the F accumulators.  Disjoint writes
                    # after the start land on pending-zero bytes.
                    nc.tensor.matmul(pE[:, 256 * bh + 8 * pl:
                                        256 * bh + 8 * (pl + 1)], sl, emat,
                                     start=(j == 0), stop=(j == 63))
                    nc.tensor.matmul(pF[:, CH * bh:CH * (bh + 1)], sl, fmat,
                                     start=(j == 0), stop=(j == 63))
                v_t = vpool.tile([128, 2 * B], F16, name="V", tag="V")
                with nc.allow_low_precision(reason="vpx f16"):
                    nc.vector.tensor_copy(out=v_t, in_=pE)
                V = [v_t[:, 0:B], v_t[:, B:2 * B]]
                # transpose V chunks -> U[pc][128 p', 256 b]
                U = [vpool.tile([128, B], F16, name=f"U{pc}", tag=f"U{pc}")
                     for pc in range(2)]
                ptt = pssm.tile([128, 1024], F16, name="ptt", tag="pst",
                                bufs=1)
                for pc in range(2):
                    for bh in range(2):
                        tps = ptt[:, 128 * (2 * pc + bh):
                                  128 * (2 * pc + bh + 1)]
                        nc.tensor.transpose(
                            tps, V[bh][:, 128 * pc:128 * (pc + 1)], ident16)
                    nc.vector.tensor_copy(
                        out=U[pc], in_=ptt[:, 256 * pc:256 * (pc + 1)])
                # vch: copy + transpose into vchT rows m3..m3+16
                c_s = spool.tile([128, 2 * CH], F16, name="c_s", tag="c_s")
                with nc.allow_low_precision(reason="vch f16"):
                    nc.vector.tensor_copy(out=c_s, in_=pF)
                for bh in range(2):
                    tpc = ptt[0:CH, 512 + 128 * bh:512 + 128 * (bh + 1)]
                    nc.tensor.transpose(tpc, c_s[:, CH * bh:CH * (bh + 1)],
                                        ident16)
                nc.vector.tensor_copy(
                    out=vchT[m3:m3 + CH, t3, :], in_=ptt[0:CH, 512:768])
                stage1_out[slot] = U

            def stage2(slot, oi):
                m3 = 32 * (slot % 3)
                t3 = slot // 3
                U = stage1_out.pop(slot)
                # -------- psi mid transfer matrices
                wp = wp_tiles.pop(slot)
                mst = mpool.tile([128, 2, BD * BD], F16, name="mst", tag="mst")
                # 16 psum chunks per slot ([128,512], bufs=4) so the PE can
                # run several chunks ahead of the Act/DVE drains
                for bh in range(2):
                    for n in range(8):
                        ps = psmm.tile([128, 512], F32, name="ps",
                                       tag="ps_mm", bufs=3)
                        cs = slice(n * 512, (n + 1) * 512)
                        nc.tensor.matmul(
                            ps, U[0][:, bh * 128:(bh + 1) * 128],
                            wp[:, 0, cs], start=True, stop=False)
                        nc.tensor.matmul(
                            ps, U[1][:, bh * 128:(bh + 1) * 128],
                            wp[:, 1, cs], start=False, stop=True)
                        with nc.allow_low_precision(reason="m f16"):
                            if (n + bh) % 2:
                                nc.scalar.copy(
                                    mst[:, bh, cs], ps)
                            else:
                                nc.vector.tensor_copy(
                                    out=mst[:, bh, cs], in_=ps)
                    if oi == SLOTS - 1:
                        mqs = ([nc.gpsimd, nc.scalar] if bh == 0 else
                               [nc.sync, nc.gpsimd])
                    else:
                        mqs = ([nc.gpsimd, nc.gpsimd] if bh == 0 else
                               [nc.sync, nc.sync])
                    for hh in range(2):
                        mqs[hh].dma_start(
                            out=mpsi_out[slot, bh * 128:(bh + 1) * 128,
                                         2048 * hh:2048 * (hh + 1)],
                            in_=mst[:, bh, 2048 * hh:2048 * (hh + 1)])

                # -------- chi mid transfer matrices
                mstc = mpool.tile([128, 2, RC * RC], F16, name="mstc",
                                  tag="mstc")
                for bh in range(2):
                    psc = pschi.tile([128, 1024], F32, name="psc",
                                     tag="ps_chi", bufs=1)
                    for n in range(2):
                        nc.tensor.matmul(
                            psc[:, n * 512:(n + 1) * 512],
                            vchT[m3:m3 + CH, t3, bh * 128:(bh + 1) * 128],
                            wc_t[m3:m3 + CH, t3, n * 512:(n + 1) * 512],
                            start=True, stop=True)
                    with nc.allow_low_precision(reason="mc f16"):
                        if bh:
                            nc.scalar.copy(mstc[:, bh, :], psc)
                        else:
                            nc.vector.tensor_copy(out=mstc[:, bh, :], in_=psc)
                nc.sync.dma_start(out=mchi_out[slot].rearrange(
                    "(c b) f -> b c f", c=2), in_=mstc)

                # -------- boundary contractions (host keeps core0/core7 only)
                if slot in (0, SLOTS - 1):
                    # base-0 copy of this slot's vchT for the boundary matmuls
                    vch0 = spool.tile([CH, B], F16, name="vch0", tag="vch0")
                    nc.gpsimd.tensor_copy(out=vch0,
                                          in_=vchT[m3:m3 + CH, t3, :])
                    wpt = wfp_t if slot == 0 else wlp_t
                    pout = v0p_out if slot == 0 else wlast_out
                    cout = v0c_out if slot == 0 else tchi_out
                    wct_b = wfc_t if slot == 0 else wlc_t
                    cw = RC if slot == 0 else RC * OUT
                    for bh in range(2):
                        psbt = psmm.tile([128, 512], F32, name="ps",
                                         tag="ps_mm", bufs=3)
                        psb = psbt[:, 0:BD]
                        for k in range(2):
                            nc.tensor.matmul(psb,
                                             U[k][:, bh * 128:(bh + 1) * 128],
                                             wpt[:, k * BD:(k + 1) * BD],
                                             start=(k == 0), stop=(k == 1))
                        v0s = spool.tile([128, BD], F32, name="v0s", tag="bnd")
                        nc.vector.tensor_copy(out=v0s, in_=psb)
                        nc.gpsimd.dma_start(out=pout[bh * 128:(bh + 1) * 128, :],
                                            in_=v0s)
                        psct = psmm.tile([128, 512], F32, name="ps",
                                         tag="ps_mm", bufs=3)
                        psc0 = psct[:, 0:cw]
                        nc.tensor.matmul(psc0, vch0[:, bh * 128:(bh + 1) * 128],
                                         wct_b, start=True, stop=True)
                        v0cs = spool.tile([128, cw], F32, name="v0cs",
                                          tag="bndc")
                        nc.vector.tensor_copy(out=v0cs, in_=psc0)
                        nc.gpsimd.dma_start(out=cout[bh * 128:(bh + 1) * 128, :],
                                            in_=v0cs)

            stage1(ORDER[0], 0)
            for oi in range(SLOTS):
                if oi + 1 < SLOTS:
                    stage1(ORDER[oi + 1], oi + 1)
                stage2(ORDER[oi], oi)
    nc.finalize()
    return nc


# ---------------------------------------------------------------- launch B
def build_launch_b():
    """Batch-sharded chains as four per-batch stationary-matvec streams.

    Each stream holds its state as an f16 [bond, width] SBUF tile whose
    partition base cycles with the site index (psi: 0/64; chi: 0/32/64),
    matching where the host packed that site's stationary matrix in its
    DMA tile (matmul requires lhsT/rhs/psum bases to agree and be in
    {0,32,64}).  A site = `width` single-column matmuls (one per batch,
    PSUM column out) + one PSUM->SBUF f16 state copy (DVE/Act).  The psi
    streams carry the batch-independent phi chain as column 32.  The chi
    bwd stream carries a matrix state (32l x 10o per batch).  Finals: the
    psi/phi dot via a ones-matmul partition reduce; chi fwd/bwd per-batch
    dots to [10, 32b], transposed and scaled by psi*phi on the DVE.
    """
    nc = bacc.Bacc("TRN2", target_bir_lowering=False, debug=False,
                   num_devices=NCORES)
    mpf_in = nc.dram_tensor("mpf", [PTF, 128, BW * BD], F16, kind="ExternalInput").ap()
    mpb_in = nc.dram_tensor("mpb", [PTB, 128, BW * BD], F16, kind="ExternalInput").ap()
    mcf_in = nc.dram_tensor("mcf", [CTF, 96, BSH * RC], F16, kind="ExternalInput").ap()
    mcb_in = nc.dram_tensor("mcb", [CTB, 96, BSH * RC], F16, kind="ExternalInput").ap()
    # packed initial states: cols 0:33 v0pT|phi0, 33:66 wlT|phiL (rows 0:64);
    # cols 66:98 v0cT (rows 0:32), cols 98:418 tT (rows 0:32)
    NI = 2 * BW + BSH + BSH * OUT
    init_in = nc.dram_tensor("init", [BD, NI], F16, kind="ExternalInput").ap()
    ident_in = nc.dram_tensor("ident", [RC, RC], F32, kind="ExternalInput").ap()

    out_out = nc.dram_tensor("out", [BSH, OUT], F32, kind="ExternalOutput").ap()

    with tile.TileContext(nc) as tc:
        with (
            tc.tile_pool(name="consts", bufs=1) as cpool,
            tc.tile_pool(name="mload", bufs=2) as mpool,
            tc.tile_pool(name="states", bufs=2) as spool,
            tc.tile_pool(name="psA", bufs=1, space="PSUM") as psA,
            tc.tile_pool(name="psB", bufs=1, space="PSUM") as psB,
        ):
            ident_t = cpool.tile([RC, RC], F32, name="ident_t")
            nc.gpsimd.dma_start(out=ident_t, in_=ident_in)
            ones32 = cpool.tile([128, 1], F32, name="ones32")
            nc.vector.memset(ones32, 1.0)
            onesw = cpool.tile([128, BSH], F32, name="onesw")
            nc.vector.memset(onesw, 1.0)

            # stream initial states, one packed DMA
            init_t = cpool.tile([BD, NI], F16, name="init_t")
            nc.sync.dma_start(out=init_t, in_=init_in)
            stf = init_t[0:BD, 0:BW]
            stb = init_t[0:BD, BW:2 * BW]
            stc = init_t[0:RC, 2 * BW:2 * BW + BSH]
            stg = init_t[0:RC, 2 * BW + BSH:NI]

            # group DMA tiles for the four streams: one shared weighted
            # rotation over the three DMA-capable queues (DVE can't DMA);
            # Act gets a lighter share since it also drains chi-bwd states
            ROT = [nc.sync, nc.gpsimd, nc.scalar, nc.sync, nc.gpsimd,
                   nc.scalar, nc.sync, nc.gpsimd]
            _gctr = {"n": 0}

            def load_group(tag, dram, t0, ntiles, width):
                gt = mpool.tile([dram.shape[1], ntiles, width], F16,
                                name=f"g_{tag}", tag=f"g_{tag}", bufs=4)
                q = ROT[_gctr["n"] % len(ROT)]
                _gctr["n"] += 1
                q.dma_start(
                    out=gt, in_=dram[t0:t0 + ntiles].rearrange("t p f -> p t f"))
                return gt

            # Each stream is a generator yielding once per site so the four
            # chains can be emitted interleaved (round-robin): the PE executes
            # its queue in program order, so sequential emission would
            # serialize the streams' latencies.
            def stream_steps(tag, dram, nsites, state, ps_pool,
                             bond, per_tile, grp, owidth, ncols, result,
                             copy_engines):
                gt = None
                ntiles_tot = (nsites + per_tile - 1) // per_tile
                # group boundaries: first group small (2) so the stream can
                # start as soon as possible; then groups of `grp`
                bounds = [0, min(1, ntiles_tot)]
                while bounds[-1] < ntiles_tot:
                    bounds.append(min(bounds[-1] + grp, ntiles_tot))
                tile2group = {}
                for gi in range(len(bounds) - 1):
                    for t in range(bounds[gi], bounds[gi + 1]):
                        tile2group[t] = (gi, bounds[gi], t - bounds[gi])
                for s in range(nsites):
                    t_idx, off = divmod(s, per_tile)
                    gi, g0, g_off = tile2group[t_idx]
                    if t_idx == g0 and off == 0:
                        n = bounds[gi + 1] - g0
                        gt = load_group(tag, dram, g0, n, ncols * bond)
                    base = bond * off
                    nbase = bond * ((s + 1) % per_tile)
                    ps = ps_pool.tile([128, ncols * owidth], F32,
                                      name=f"ps_{tag}", tag=f"ps_{tag}",
                                      bufs=1)
                    for b in range(ncols):
                        nc.tensor.matmul(
                            ps[nbase:nbase + bond, owidth * b:owidth * (b + 1)],
                            gt[base:base + bond, g_off,
                               bond * b:bond * (b + 1)],
                            state[base:base + bond,
                                  owidth * b:owidth * (b + 1)],
                            start=True, stop=True)
                    state = spool.tile([128, ncols * owidth], F16,
                                       name=f"st_{tag}", tag=tag)
                    ceng = copy_engines[s % len(copy_engines)]
                    with nc.allow_low_precision(reason="f16 chain state"):
                        if ceng is nc.scalar:
                            ceng.copy(state[nbase:nbase + bond, :],
                                      ps[nbase:nbase + bond, :])
                        else:
                            ceng.tensor_copy(out=state[nbase:nbase + bond, :],
                                             in_=ps[nbase:nbase + bond, :])
                    yield
                result.append(state)

            res_f, res_b, res_c, res_g = [], [], [], []
            gens = [
                stream_steps("stf", mpf_in, NPF, stf, psA,
                             BD, 2, PGRP, 1, BW, res_f, [nc.vector]),
                stream_steps("stb", mpb_in, NPB, stb, psA,
                             BD, 2, PGRP, 1, BW, res_b, [nc.vector]),
                stream_steps("stc", mcf_in, NCF, stc, psB,
                             RC, 3, CGRP, 1, BSH, res_c, [nc.vector]),
                stream_steps("stg", mcb_in, NCB, stg, psB,
                             RC, 3, CGRP, OUT, BSH, res_g,
                             [nc.scalar, nc.vector]),
            ]
            live = list(gens)
            while live:
                for g in list(live):
                    try:
                        next(g)
                    except StopIteration:
                        live.remove(g)
            stf, stb, stc, stg = res_f[0], res_b[0], res_c[0], res_g[0]

            fb_f = BD * (NPF % 2)   # 0
            fb_b = BD * (NPB % 2)   # 0
            fb_c = RC * (NCF % 3)   # 32
            fb_g = RC * (NCB % 3)   # 32

            # psi_val[b] = sum_l stf[l,b]*stb[l,b]; col 32 gives phi_val.
            # f32 throughout: the products are ~1e-8 and underflow in f16.
            prod = spool.tile([128, BW], F32, name="prod", tag="prod")
            nc.vector.tensor_tensor(out=prod[fb_f:fb_f + BD, :],
                                    in0=stf[fb_f:fb_f + BD, :],
                                    in1=stb[fb_b:fb_b + BD, :],
                                    op=MULT)
            ppv = psA.tile([BW, 1], F32, name="ppv", tag="ppv", bufs=1)
            nc.tensor.matmul(ppv, prod[fb_f:fb_f + BD, :],
                             ones32[fb_f:fb_f + BD, :], start=True, stop=True)
            psiphi = spool.tile([BW, 1], F32, name="psiphi", tag="fin")
            nc.vector.tensor_copy(out=psiphi, in_=ppv)
            # replicate phival (row 32) across the 32 batch partitions via a
            # ones-row matmul (bases must agree at 32), then fold into psival
            prep = psA.tile([BSH, 1], F32, name="prep", tag="prep", bufs=1)
            nc.tensor.matmul(prep, onesw[32:33, :], psiphi[BSH:BW, 0:1],
                             start=True, stop=True)
            scal = spool.tile([BSH, 1], F32, name="scal", tag="fin2c")
            nc.vector.tensor_tensor(out=scal, in0=psiphi[0:BSH, :],
                                    in1=prep, op=MULT)

            # chi_out[o,b] = sum_l stg[l, b*OUT+o] * stc[l, b]
            pcf = psB.tile([OUT, BSH], F32, name="pcf", tag="pcf", bufs=1)
            for b in range(BSH):
                nc.tensor.matmul(pcf[:, b:b + 1],
                                 stg[fb_g:fb_g + RC, OUT * b:OUT * (b + 1)],
                                 stc[fb_c:fb_c + RC, b:b + 1],
                                 start=True, stop=True)
            chifs = spool.tile([OUT, BSH], F32, name="chifs", tag="fin2")
            nc.vector.tensor_copy(out=chifs, in_=pcf)
            pt = psA.tile([BSH, OUT], F32, name="pt", tag="pt", bufs=1)
            nc.tensor.transpose(pt, chifs, ident_t[0:OUT, 0:OUT])
            res = spool.tile([BSH, OUT], F32, name="res", tag="fin3")
            nc.vector.tensor_scalar_mul(out=res, in0=pt, scalar1=scal)
            nc.sync.dma_start(out=out_out, in_=res)
    nc.finalize()
    return nc


# ------------------------------------------------------------- host glue
_cache = {}
LAST_RESULTS = []  # [(label, BassKernelResults)] from the most recent kernel()
LAST_INMAPS = {}   # {"a": in_maps_a, "b": in_maps_b} from the most recent kernel()


def _prep_inputs_a(inputs):
    # f16 upload of x: the on-device reductions accumulate in f32; the
    # 0.05% per-element cast error is far below the f16 weight error.
    x = np.asarray(inputs["x"], dtype=np.float32)
    # [site, (ch, p_hi), (p_lo, b)]: x[b, site, p, c] with p = ph*32 + pl
    xe = np.ascontiguousarray(
        x.transpose(1, 3, 2, 0)                       # (site, c, p, b)
        .reshape(PAT, CH, 8, 32, B)                   # (site, c, ph, pl, b)
        .reshape(PAT, 128, 32 * B).astype(np.float16))

    # psi_mid (62,l,r,p) -> (62, p', l*r), rows p'-permuted (no /CH: E has it)
    pm = inputs["psi_mid"].astype(np.float32)
    wpsi = np.ascontiguousarray(
        pm.transpose(0, 3, 1, 2).reshape(NMID, PIX, BD * BD)[:, PPERM, :])
    # chi_mid (62,l,r,ch) -> (62, ch, rc*rc) (no /PIX: F has it)
    cm = inputs["chi_mid"].astype(np.float32)
    wchi = np.ascontiguousarray(
        cm.transpose(0, 3, 1, 2).reshape(NMID, CH, RC * RC))

    wfp = np.ascontiguousarray(
        inputs["psi_first"].T.astype(np.float32)[PPERM, :]).astype(np.float16)
    wlp = np.ascontiguousarray(
        inputs["psi_last"].T.astype(np.float32)[PPERM, :]).astype(np.float16)
    wfc = np.ascontiguousarray(
        inputs["chi_first"].T.astype(np.float32)).astype(np.float16)
    wlc = np.ascontiguousarray(
        inputs["chi_last"].astype(np.float32).transpose(1, 0, 2)
        .reshape(CH, RC * OUT)).astype(np.float16)

    ident = np.eye(128, dtype=np.float16)

    # selection matrices: rows (c, ph); E sums over c (vpx = mean_c),
    # F sums over ph (+ the p_lo accumulation group) for vch = mean_p.
    emat = np.zeros((128, 8), np.float16)
    fmat = np.zeros((128, CH), np.float16)
    for q in range(128):
        c, ph = q // 8, q % 8
        emat[q, ph] = 1.0 / CH
        fmat[q, c] = 1.0 / PIX

    zero_pw = np.zeros_like(wpsi[0])
    zero_cw = np.zeros_like(wchi[0])
    in_maps = []
    for k in range(NCORES):
        # slot j of core k handles patch 8k+j; mid site s uses weight s-1
        wp_slots = np.stack([
            wpsi[8 * k + j - 1] if 1 <= 8 * k + j <= NMID else zero_pw
            for j in range(SLOTS)]).astype(np.float16)
        wc_slots = np.stack([
            wchi[8 * k + j - 1] if 1 <= 8 * k + j <= NMID else zero_cw
            for j in range(SLOTS)]).astype(np.float16)
        # pack chi weights: tile t rows 32*m..+16 = slot 3t+m
        wc_packed = np.zeros((3, 96, RC * RC), np.float16)
        for j in range(SLOTS):
            wc_packed[j // 3, 32 * (j % 3):32 * (j % 3) + CH] = wc_slots[j]
        z = np.zeros
        in_maps.append({
            "xe": np.ascontiguousarray(xe[8 * k:8 * (k + 1)]),
            "wpsi": np.ascontiguousarray(wp_slots),
            "wchi": wc_packed,
            "emat": emat,
            "fmat": fmat,
            "wfp": wfp if k == 0 else z((PIX, BD), np.float16),
            "wlp": wlp if k == NCORES - 1 else z((PIX, BD), np.float16),
            "wfc": wfc if k == 0 else z((CH, RC), np.float16),
            "wlc": wlc if k == NCORES - 1 else z((CH, RC * OUT), np.float16),
            "ident": ident,
        })
    return in_maps


def _assemble_m(results_a):
    mp_parts, mc_parts = [], []
    for k in range(NCORES):
        lo = 1 if k == 0 else 0
        hi = SLOTS - 1 if k == NCORES - 1 else SLOTS
        mp_parts.append(results_a[k]["mpsi"][lo:hi])
        mc_parts.append(results_a[k]["mchi"][lo:hi])
    mp_full = np.concatenate(mp_parts).reshape(NMID, B, BD, BD)
    mc_full = np.concatenate(mc_parts).reshape(NMID, B, RC, RC)
    return mp_full, mc_full


def _pack_psi(arr):
    """(nsites, l_or_r(64), 33, 64) site-major -> (ntiles, 128, 33*64)."""
    n = arr.shape[0]
    return np.ascontiguousarray(
        arr.reshape(n // 2, 2 * BD, BW * BD))


def _pack_chi(arr, ntiles):
    """(nsites, 32, 32, 32) -> (ntiles, 96, 1024) with zero pad."""
    n = arr.shape[0]
    out = np.zeros((ntiles, 3, RC, BSH * RC), arr.dtype)
    flat = arr.reshape(n, RC, BSH * RC)
    for s in range(n):
        out[s // 3, s % 3] = flat[s]
    return np.ascontiguousarray(out.reshape(ntiles, 3 * RC, BSH * RC))


def _prep_inputs_b(res_a, inputs):
    mp_full, mc_full = _assemble_m(res_a)   # (62,256,64,64), (62,256,32,32)
    v0p, v0c = res_a[0]["v0p"], res_a[0]["v0c"]
    wlast = res_a[NCORES - 1]["wlast"]
    tchi = res_a[NCORES - 1]["tchi"]
    ident = np.eye(RC, dtype=np.float32)
    # phi chain matrices (batch-independent): mid i uses phi_mid[i][:,:,i+1]
    phiw = np.stack([np.asarray(inputs["phi_mid"])[i][:, :, i + 1]
                     for i in range(NMID)]).astype(np.float32)
    phif0 = np.asarray(inputs["phi_first"])[:, 0].astype(np.float32)
    phil63 = np.asarray(inputs["phi_last"])[:, 63].astype(np.float32)
    in_maps_b = []
    for j in range(NCORES):
        sl = slice(BSH * j, BSH * (j + 1))
        # psi fwd: mids 0..31 as (site, l, b, r); phi rides as column 32
        pf = np.zeros((NPF, BD, BW, BD), np.float32)
        pf[:, :, 0:BSH, :] = mp_full[0:NPF, sl].transpose(0, 2, 1, 3)
        pf[:, :, BSH, :] = phiw[0:NPF]
        mpf = _pack_psi(pf.astype(np.float16))
        # psi bwd: mids 61..32 descending as (site, r, b, l); phi transposed
        pb = np.zeros((NPB, BD, BW, BD), np.float32)
        pb[:, :, 0:BSH, :] = (mp_full[NMID - 1:NMID - 1 - NPB:-1, sl]
                              .transpose(0, 3, 1, 2))
        pb[:, :, BSH, :] = phiw[NMID - 1:NMID - 1 - NPB:-1].transpose(0, 2, 1)
        mpb = _pack_psi(pb.astype(np.float16))
        # chi fwd: mids 0..30 as (site, l, b, r)
        mcf = _pack_chi(mc_full[0:NCF, sl].transpose(0, 2, 1, 3)
                        .astype(np.float16), CTF)
        # chi bwd: mids 61..31 descending as (site, r, b, l)
        mcb = _pack_chi(mc_full[NMID - 1:NMID - 1 - NCB:-1, sl]
                        .transpose(0, 3, 1, 2).astype(np.float16), CTB)
        tT = (tchi[sl].reshape(BSH, RC, OUT).transpose(1, 0, 2)
              .reshape(RC, BSH * OUT))
        NI = 2 * BW + BSH + BSH * OUT
        init = np.zeros((BD, NI), np.float16)
        init[0:BD, 0:BSH] = v0p[sl].T.astype(np.float16)
        init[0:BD, BSH] = phif0.astype(np.float16)
        init[0:BD, BW:BW + BSH] = wlast[sl].T.astype(np.float16)
        init[0:BD, BW + BSH] = phil63.astype(np.float16)
        init[0:RC, 2 * BW:2 * BW + BSH] = v0c[sl].T.astype(np.float16)
        init[0:RC, 2 * BW + BSH:] = tT.astype(np.float16)
        in_maps_b.append({
            "mpf": mpf, "mpb": mpb, "mcf": mcf, "mcb": mcb,
            "init": np.ascontiguousarray(init),
            "ident": ident,
        })
    return in_maps_b


def kernel(**inputs):
    core_ids = list(range(NCORES))
    if "nca" not in _cache:
        _cache["nca"] = build_launch_a()
        _cache["ncb"] = build_launch_b()
    nca, ncb = _cache["nca"], _cache["ncb"]

    LAST_RESULTS.clear()
    in_maps_a = _prep_inputs_a(inputs)
    LAST_INMAPS["a"] = in_maps_a
    bkr_a = run_bass_kernel_spmd(nca, in_maps_a, core_ids=core_ids)
    LAST_RESULTS.append(("launch_a", bkr_a))
    res_a = bkr_a.results

    in_maps_b = _prep_inputs_b(res_a, inputs)
    LAST_INMAPS["b"] = in_maps_b
    bkr_b = run_bass_kernel_spmd(ncb, in_maps_b, core_ids=core_ids)
    LAST_RESULTS.append(("launch_b", bkr_b))
    res_b = bkr_b.results

    out = np.empty((B, OUT), np.float32)
    for j in range(NCORES):
        out[BSH * j:BSH * (j + 1)] = res_b[j]["out"]
    return out
